# revision 1
# baseline (speedup 1.0000x reference)
"""Trainium2 Bass kernel for nn_DynamicConv (dense_cnn).

out[i, j, co, h, w] = sum_k (conv_k(x_i)[co, h, w] + b_k[co]) * attn[j, k]
attn = softmax(softmax(MLP(meanpool(x)), k) / TAU, k)

Sharding: data-parallel over batch i across 8 cores.  Each core convolves its
own sample (9 shifted matmuls over a zero-padded image, contraction = CIN=128,
fp32r) and computes the full [B, K] attention matrix locally from a replicated
copy of x (it is tiny), then applies the cross-batch blend as one
block-diagonal matmul per 16-channel group:
  contraction 64 = (k=4) x (co16), M = 128 = (j=8) x (co16).
Conv weights are host-packed so output channels land in (co, k)-interleaved
partition order, which makes the blend's rhs a contiguous partition range.
All matmul operands are float32r (FP22 multiply, fp32 accumulate) — full PE
rate; the BIR verifier requires producers of those tiles to emit float32r.
"""

import sys

import numpy as np

if "/opt/trn_rl_repo" not in sys.path:
    sys.path.insert(0, "/opt/trn_rl_repo")

import concourse.bacc as bacc
import concourse.bass as bass
import concourse.mybir as mybir
import concourse.tile as tile

F32 = mybir.dt.float32
F32R = mybir.dt.float32r
AF = mybir.ActivationFunctionType
AX = mybir.AxisListType
ALU = mybir.AluOpType

B = 8
CIN = 128
COUT = 256
K = 4
KS = 3
HW = 48
HW2 = HW * HW          # 2304
WP = HW + 2            # 50 (padded)
HID = 256
TAU = 30.0
NCORES = 8

ROW_GROUPS = [(0, 10), (10, 10), (20, 10), (30, 10), (40, 8)]
CHUNKS = [(0, 512), (512, 512), (1024, 512), (1536, 512), (2048, 256)]


def build_nc():
    nc = bacc.Bacc("TRN2", debug=False, num_devices=NCORES)

    xi = nc.dram_tensor("xi", [CIN, HW2], F32R, kind="ExternalInput").ap()
    # [ci, t, tap, p] flattened; p = c*4 + k encodes (co = 32 t + c, k)
    wconv = nc.dram_tensor(
        "wconv", [CIN, 8 * 9 * 128], F32R, kind="ExternalInput"
    ).ap()
    bconv = nc.dram_tensor("bconv", [128, 8], F32, kind="ExternalInput").ap()
    w1t = nc.dram_tensor("w1t", [CIN, HID], F32R, kind="ExternalInput").ap()
    b1c = nc.dram_tensor("b1c", [128, 2], F32, kind="ExternalInput").ap()
    w2t = nc.dram_tensor("w2t", [128, 2 * K], F32R, kind="ExternalInput").ap()
    b2r = nc.dram_tensor("b2r", [1, K], F32R, kind="ExternalInput").ap()
    ident8 = nc.dram_tensor("ident8", [B, B], F32R, kind="ExternalInput").ap()
    # memset can't write float32r tiles (walrus ISA check) — ship constants
    zer128 = nc.dram_tensor("zer128", [128, 128], F32R, kind="ExternalInput").ap()
    one18 = nc.dram_tensor("one18", [1, B], F32R, kind="ExternalInput").ap()
    out = nc.dram_tensor("out", [B, COUT, HW2], F32, kind="ExternalOutput").ap()
    # internal DRAM for the cross-core attention-row AllGather
    cc_in = nc.dram_tensor("cc_in", [1, K], F32).ap()
    cc_out = nc.dram_tensor("cc_out", [B, K], F32, addr_space="Shared").ap()

    with tile.TileContext(nc, num_cores=NCORES) as tc:
        with (
            tc.tile_pool(name="const", bufs=1) as const,
            tc.tile_pool(name="csb", bufs=8) as csb_pool,
            tc.tile_pool(name="osb", bufs=5) as osb_pool,
            tc.tile_pool(name="psA", bufs=3, space="PSUM") as psA,
            tc.tile_pool(name="psB", bufs=4, space="PSUM") as psB,
            tc.tile_pool(name="psM", bufs=1, space="PSUM") as psM,
        ):
            # ---- conv-critical loads first: image, then weights ----
            # each queue moves ~42GB/s (one descriptor per partition line), so
            # split large transfers across queues
            xfull = const.tile([128, HW2], F32R)
            nc.gpsimd.dma_start(xfull[:], xi[:, :])
            ztile = const.tile([128, 128], F32R)
            nc.sync.dma_start(ztile[:], zer128[:, :])

            # pre-warm the ACT function tables (1.3us each if loaded lazily
            # inside the latency-critical chains)
            actw = const.tile([128, 1], F32)
            zcol = ztile[:, 0:1].bitcast(F32)
            nc.scalar.activation(actw[:], zcol, AF.Identity, bias=zcol)
            nc.scalar.activation(actw[:], zcol, AF.Relu, bias=zcol)
            nc.scalar.activation(actw[:], zcol, AF.Exp, bias=zcol)
            nc.scalar.copy(actw[:], zcol)

            # padded image built on-chip (a strided DMA here would shatter
            # into 192B descriptors and swamp the queues)
            xp = const.tile([128, WP * WP], F32R)
            xp3 = xp[:].rearrange("p (h w) -> p h w", w=WP)
            xf3 = xfull[:].rearrange("p (h w) -> p h w", w=HW)
            nc.vector.tensor_copy(xp3[:, 1 : 1 + HW, 1 : 1 + HW], xf3[:, :, :])
            nc.vector.tensor_copy(xp3[:, 0, 0:WP], ztile[:, 0:WP])
            nc.vector.tensor_copy(xp3[:, WP - 1, 0:WP], ztile[:, 0:WP])
            nc.vector.tensor_copy(xp3[:, 1 : 1 + HW, 0], ztile[:, 0:HW])
            nc.vector.tensor_copy(xp3[:, 1 : 1 + HW, WP - 1], ztile[:, 0:HW])

            wt = []
            for t in range(8):
                w = const.tile([128, 9 * 128], F32R, tag=f"wt{t}")
                nc.gpsimd.dma_start(w[:], wconv[:, t * 9 * 128 : (t + 1) * 9 * 128])
                wt.append(w)
            bct = const.tile([128, 8], F32)
            nc.gpsimd.dma_start(bct[:], bconv[:, :])
            w1s = const.tile([128, HID], F32R)
            nc.gpsimd.dma_start(w1s[:], w1t[:, :])
            b1s = const.tile([128, 2], F32)
            nc.gpsimd.dma_start(b1s[:], b1c[:, :])
            w2s = const.tile([128, 2 * K], F32R)
            nc.gpsimd.dma_start(w2s[:], w2t[:, :])
            b2s = const.tile([1, K], F32R)
            nc.gpsimd.dma_start(b2s[:], b2r[:, :])
            id8 = const.tile([B, B], F32R)
            nc.gpsimd.dma_start(id8[:], ident8[:, :])
            ones = const.tile([1, B], F32R)
            nc.gpsimd.dma_start(ones[:], one18[:, :])

            # ---- local global-average pooling (own sample only) ----
            pooled_loc = const.tile([128, 1], F32R)  # [ci, 1] sums; 1/HW2 in w1t
            with nc.allow_low_precision(reason="fp32r matmul operand"):
                nc.vector.tensor_reduce(
                    pooled_loc[:], xfull[:], axis=AX.X, op=ALU.add
                )

            cs_tiles = [None] * 8

            def emit_conv(t):
                cs = csb_pool.tile([128, HW2], F32R, tag="csb")
                cs_tiles[t] = cs
                for (r0, R) in ROW_GROUPS:
                    pt = psA.tile([128, R * HW], F32, tag="cps")
                    for tap in range(9):
                        dh, dw = divmod(tap, 3)
                        rhs = xp3[:, r0 + dh : r0 + dh + R, dw : dw + HW]
                        nc.tensor.matmul(
                            pt[:],
                            lhsT=wt[t][:, tap * 128 : (tap + 1) * 128],
                            rhs=rhs,
                            start=(tap == 0),
                            stop=(tap == 8),
                        )
                    # PSUM -> SBUF eviction, fused with the conv bias add
                    nc.scalar.activation(
                        cs[:, r0 * HW : (r0 + R) * HW],
                        pt[:],
                        AF.Identity,
                        bias=bct[:, t : t + 1],
                    )

            def emit_blend(t, BD):
                cs = cs_tiles[t]
                for u in range(2):
                    g = 2 * t + u
                    ob = osb_pool.tile([128, HW2], F32, tag="osb")
                    for ci_, (c0, C) in enumerate(CHUNKS):
                        bp = psB.tile([128, C], F32, tag="bps")
                        nc.tensor.matmul(
                            bp[:],
                            lhsT=BD[:, 128 * u : 128 * u + 128],
                            rhs=cs[:, c0 : c0 + C],
                            start=True,
                            stop=True,
                        )
                        # PSUM drain balanced across DVE and ACT so psB bank
                        # recycling (not one engine) sets the blend rate
                        if ci_ in (1, 4):
                            nc.scalar.copy(ob[:, c0 : c0 + C], bp[:])
                        else:
                            nc.vector.tensor_copy(ob[:, c0 : c0 + C], bp[:])
                    nc.gpsimd.dma_start(out[:, 16 * g : 16 * g + 16, :], ob[:])

            # ---- attention MLP + double softmax (local row, computed with
            # the same batched shapes that are known to compile: pooled is
            # broadcast to 8 columns, giving 8 identical rows) ----
            pooled8 = const.tile([128, B], F32R)
            nc.vector.tensor_copy(
                pooled8[:], pooled_loc[:, 0:1].broadcast_to([128, B])
            )
            hd = []
            for h in range(2):
                hps = psM.tile([128, B], F32, tag="mlp")
                nc.tensor.matmul(
                    hps[:],
                    lhsT=w1s[:, h * 128 : (h + 1) * 128],
                    rhs=pooled8[:],
                    start=True,
                    stop=True,
                )
                hsb = const.tile([128, B], F32R, tag=f"hd{h}")
                nc.scalar.activation(hsb[:], hps[:], AF.Relu, bias=b1s[:, h : h + 1])
                hd.append(hsb)

            lps = psM.tile([B, K], F32, tag="mlp")
            nc.tensor.matmul(
                lps[:], lhsT=hd[0][:], rhs=w2s[:, 0:K], start=True, stop=False
            )
            nc.tensor.matmul(
                lps[:], lhsT=hd[1][:], rhs=w2s[:, K : 2 * K], start=False, stop=False
            )
            nc.tensor.matmul(
                lps[:], lhsT=ones[:], rhs=b2s[:], start=False, stop=True
            )

            # double softmax over k (shift-invariant: max-subtraction dropped)
            e1 = const.tile([B, K], F32)
            nc.scalar.activation(e1[:], lps[:], AF.Exp, bias=0.0, scale=1.0)
            s1 = const.tile([B, 1], F32)
            nc.vector.tensor_reduce(s1[:], e1[:], axis=AX.X, op=ALU.add)
            r1 = const.tile([B, 1], F32)
            nc.vector.reciprocal(r1[:], s1[:])
            a1 = const.tile([B, K], F32)
            nc.vector.tensor_scalar_mul(a1[:], e1[:], r1[:, 0:1])

            e2 = const.tile([B, K], F32)
            nc.scalar.activation(e2[:], a1[:], AF.Exp, bias=0.0, scale=1.0 / TAU)
            s2 = const.tile([B, 1], F32)
            nc.vector.tensor_reduce(s2[:], e2[:], axis=AX.X, op=ALU.add)
            r2 = const.tile([B, 1], F32)
            nc.vector.reciprocal(r2[:], s2[:])
            attn_loc = const.tile([B, K], F32R)
            nc.vector.tensor_scalar_mul(attn_loc[:], e2[:], r2[:, 0:1])

            # AllGather row 0 of the (identical-row) local attn -> true [B, K]
            nc.sync.dma_start(cc_in.bitcast(F32R), attn_loc[0:1, :])
            nc.gpsimd.collective_compute(
                "AllGather",
                ALU.bypass,
                replica_groups=[list(range(NCORES))],
                ins=[cc_in],
                outs=[cc_out],
            )
            attn = const.tile([B, K], F32R)
            nc.sync.dma_start(attn[:], cc_out.bitcast(F32R))

            # conv t=0..6 on the PE while the AllGather completes (the
            # collective mesh takes ~70-90us wall; hide it under conv work)
            for _t in range(7):
                emit_conv(_t)

            # attn [j, k] -> attn_T [k, j] via PE transpose
            tps = psM.tile([K, B], F32R, tag="mlp")
            nc.tensor.transpose(tps[:], attn[:], id8[:])
            atT = const.tile([K, B], F32R)
            nc.scalar.copy(atT[:], tps[:])

            # full-contraction blend weights: BD2[:, u*128 + j*16 + c] picks
            # attn[j, k] at partition 64u + c*4 + k, zero elsewhere, so the
            # matmul contracts all 128 partitions of cs (zeros are harmless)
            # and lhsT always sits at base_partition 0
            BD2 = const.tile([128, 256], F32R)
            nc.vector.tensor_copy(BD2[:, 0:128], ztile[:])
            nc.vector.tensor_copy(BD2[:, 128:256], ztile[:])
            BDv = BD2[:].rearrange("p (u j c) -> p u j c", c=16, u=2)
            for u in range(2):
                for c in range(16):
                    # compute engines can't start at partition 4c; DMA can
                    p0 = 64 * u + c * 4
                    nc.sync.dma_start(BDv[p0 : p0 + 4, u, :, c], atT[:])
            BD = BD2

            # last conv, then drain all blends
            emit_blend(0, BD)
            emit_conv(7)
            for t in range(1, 8):
                emit_blend(t, BD)

    nc.compile()
    return nc


def pack_inputs(x, conv_w, conv_b, w1, b1, w2, b2):
    """Host-side layout packing (no arithmetic beyond constant folding of the
    mean-pool scale into w1)."""
    x = np.ascontiguousarray(x, dtype=np.float32)
    x_all = x.reshape(B, CIN, HW2)

    # conv_w [K, COUT, CIN, 3, 3] -> [ci, t, tap, p] with p = c*4 + k,
    # co = 32 t + c
    w = np.asarray(conv_w, dtype=np.float32).transpose(2, 3, 4, 0, 1)  # ci kh kw k co
    w = w.reshape(CIN, KS, KS, K, 8, 32)  # ci kh kw k t c
    w = w.transpose(0, 4, 1, 2, 5, 3)  # ci t kh kw c k
    wconv = np.ascontiguousarray(w.reshape(CIN, 8 * 9 * 128))

    bc = np.asarray(conv_b, dtype=np.float32).reshape(K, 8, 32)  # k t c
    bconv = np.ascontiguousarray(bc.transpose(1, 2, 0).reshape(8, 128).T)  # [p, t]

    w1t = np.ascontiguousarray(np.asarray(w1, dtype=np.float32).T) / float(HW2)
    b1c = np.ascontiguousarray(np.asarray(b1, dtype=np.float32).reshape(2, 128).T)
    w2T = np.asarray(w2, dtype=np.float32).T  # [256, 4]
    w2t = np.ascontiguousarray(np.concatenate([w2T[:128], w2T[128:]], axis=1))
    b2r = np.asarray(b2, dtype=np.float32).reshape(1, K)
    ident8 = np.eye(B, dtype=np.float32)

    common = dict(
        wconv=wconv, bconv=bconv, w1t=w1t, b1c=b1c,
        w2t=w2t, b2r=b2r, ident8=ident8,
        zer128=np.zeros((128, 128), dtype=np.float32),
        one18=np.ones((1, B), dtype=np.float32),
    )
    in_maps = [dict(common, xi=np.ascontiguousarray(x_all[i])) for i in range(NCORES)]
    return in_maps


def run(inputs, trace=False):
    from concourse.bass_utils import run_bass_kernel_spmd

    nc = build_nc()
    in_maps = pack_inputs(**inputs)
    res = run_bass_kernel_spmd(
        nc, in_maps, core_ids=list(range(NCORES)), trace=trace
    )
    slabs = [res.results[i]["out"] for i in range(NCORES)]
    out = np.stack(slabs, axis=0).reshape(B, B, COUT, HW, HW)
    return out, res


def kernel(**inputs) -> np.ndarray:
    out, _ = run(inputs, trace=False)
    return out



# revision 4
# speedup vs baseline: 1.0724x; 1.0724x over previous
"""Trainium2 Bass kernel for nn_DynamicConv (dense_cnn).

out[i, j, co, h, w] = sum_k (conv_k(x_i)[co, h, w] + b_k[co]) * attn[j, k]
attn = softmax(softmax(MLP(meanpool(x)), k) / TAU, k)

Sharding: data-parallel over batch i across 8 cores.  All matmuls run in
bf16 (fp32r lowers to fp32_mode=HIGH which streams at 2 cycles/column —
half PE rate; bf16 streams 1 col/cycle and gets FWL weight loads).

No collective: every core receives ALL of x in bf16 (4.7 MB), rotated so
its own sample sits at slice 0, and computes the full [B, K] attention
matrix locally (global-average pools on DVE as slices arrive).  That
removes the ~50us AllGather from the critical path entirely; the host
un-rotates the j rows of each core's output slab at gather time.

Each core convolves its own sample (9 shifted matmuls over a zero-padded
image, contraction CIN=128) and applies the cross-batch blend as one
block-diagonal matmul per 16-channel group:
  contraction 64 = (k=4) x (co16), M = 128 = (j=8) x (co16).
Conv weights are host-packed so output channels land in (co, k)-interleaved
partition order, which makes the blend's rhs a contiguous partition range.
The data-dependent blend lhsT (BD) is built on-chip in ~1.5us:
  attn -> PE transpose -> atT[4,8]
  T1[p, j] = atT[p%4, j] via a one-hot replicate matmul (K1[4,128])
  BD[p, (u,j,c)] = T1[p, j] * CMASK[p, (u,j,c)]   (one DVE mul per u-half)
DMA issue is spread across the sync/scalar HWDGE rings and the gpsimd
SWDGE ring; all 16 output-slab stores issue from the otherwise-idle Sync
engine.  PE instruction order is hand-arranged (conv0, MLP, conv1,
BD-matmuls, blend0, conv2, blend1, ...) so the PE never waits on the
attention pipeline.
"""

import sys

import numpy as np

if "/opt/trn_rl_repo" not in sys.path:
    sys.path.insert(0, "/opt/trn_rl_repo")

import concourse.bacc as bacc
import concourse.bass as bass
import concourse.mybir as mybir
import concourse.tile as tile

F32 = mybir.dt.float32
BF16 = mybir.dt.bfloat16
AF = mybir.ActivationFunctionType
AX = mybir.AxisListType
ALU = mybir.AluOpType

B = 8
CIN = 128
COUT = 256
K = 4
KS = 3
HW = 48
HW2 = HW * HW          # 2304
WP = HW + 2            # 50 (padded)
HID = 256
TAU = 30.0
NCORES = 8

ROW_GROUPS = [(0, 10), (10, 10), (20, 10), (30, 10), (40, 8)]
CHUNKS = [(0, 512), (512, 512), (1024, 512), (1536, 512), (2048, 256)]


def build_nc():
    nc = bacc.Bacc("TRN2", debug=False, num_devices=NCORES)

    # all 8 samples, rotated per-core so slice 0 is the core's own sample
    xall = nc.dram_tensor("xall", [CIN, B * HW2], BF16, kind="ExternalInput").ap()
    # [ci, t, tap, p] flattened; p = c*4 + k encodes (co = 32 t + c, k)
    wconv = nc.dram_tensor(
        "wconv", [CIN, 8 * 9 * 128], BF16, kind="ExternalInput"
    ).ap()
    bconv = nc.dram_tensor("bconv", [128, 8], F32, kind="ExternalInput").ap()
    w1t = nc.dram_tensor("w1t", [CIN, HID], BF16, kind="ExternalInput").ap()
    b1c = nc.dram_tensor("b1c", [128, 2], F32, kind="ExternalInput").ap()
    w2t = nc.dram_tensor("w2t", [128, 2 * K], BF16, kind="ExternalInput").ap()
    b2r = nc.dram_tensor("b2r", [1, K], BF16, kind="ExternalInput").ap()
    one18 = nc.dram_tensor("one18", [1, B], BF16, kind="ExternalInput").ap()
    ident8 = nc.dram_tensor("ident8", [B, B], BF16, kind="ExternalInput").ap()
    k1m = nc.dram_tensor("k1m", [K, 128], BF16, kind="ExternalInput").ap()
    cmask = nc.dram_tensor("cmask", [128, 256], BF16, kind="ExternalInput").ap()
    zerb = nc.dram_tensor("zerb", [128, 64], BF16, kind="ExternalInput").ap()
    zerf = nc.dram_tensor("zerf", [128, 8], F32, kind="ExternalInput").ap()
    out = nc.dram_tensor("out", [B, COUT, HW2], F32, kind="ExternalOutput").ap()

    with tile.TileContext(nc, num_cores=NCORES) as tc:
        with (
            tc.tile_pool(name="const", bufs=1) as const,
            tc.tile_pool(name="csb", bufs=3) as csb_pool,
            tc.tile_pool(name="osb", bufs=4) as osb_pool,
            tc.tile_pool(name="psA", bufs=3, space="PSUM") as psA,
            tc.tile_pool(name="psB", bufs=4, space="PSUM") as psB,
            tc.tile_pool(name="psM", bufs=1, space="PSUM") as psM,
        ):
            # ---- DMA issue: own sample + first weights on the Sync HWDGE
            # ring, remaining samples split across the Scalar HWDGE and
            # GpSimd SWDGE rings, weights/consts behind them ----
            zb = const.tile([128, 64], BF16)
            nc.sync.dma_start(zb[:], zerb[:, :])
            zf = const.tile([128, 8], F32)
            nc.sync.dma_start(zf[:], zerf[:, :])
            xa = const.tile([128, B * HW2], BF16)
            nc.sync.dma_start(xa[:, 0:HW2], xall[:, 0:HW2])
            wt = []
            for t in range(8):
                w = const.tile([128, 9 * 128], BF16, tag=f"wt{t}")
                wt.append(w)
            nc.sync.dma_start(wt[0][:], wconv[:, 0 : 9 * 128])
            nc.sync.dma_start(wt[1][:], wconv[:, 9 * 128 : 2 * 9 * 128])
            nc.scalar.dma_start(xa[:, HW2 : 4 * HW2], xall[:, HW2 : 4 * HW2])
            nc.gpsimd.dma_start(xa[:, 4 * HW2 : 8 * HW2], xall[:, 4 * HW2 : 8 * HW2])
            bct = const.tile([128, 8], F32)
            nc.gpsimd.dma_start(bct[:], bconv[:, :])
            w1s = const.tile([128, HID], BF16)
            nc.gpsimd.dma_start(w1s[:], w1t[:, :])
            b1s = const.tile([128, 2], F32)
            nc.gpsimd.dma_start(b1s[:], b1c[:, :])
            w2s = const.tile([128, 2 * K], BF16)
            nc.gpsimd.dma_start(w2s[:], w2t[:, :])
            b2s = const.tile([1, K], BF16)
            nc.gpsimd.dma_start(b2s[:], b2r[:, :])
            ones = const.tile([1, B], BF16)
            nc.gpsimd.dma_start(ones[:], one18[:, :])
            id8 = const.tile([B, B], BF16)
            nc.gpsimd.dma_start(id8[:], ident8[:, :])
            k1s = const.tile([K, 128], BF16)
            nc.gpsimd.dma_start(k1s[:], k1m[:, :])
            cms = const.tile([128, 256], BF16)
            nc.gpsimd.dma_start(cms[:], cmask[:, :])
            # remaining conv weights (needed one per ~9us of conv)
            for t in range(2, 8):
                eng = nc.scalar if t % 2 == 0 else nc.gpsimd
                eng.dma_start(wt[t][:], wconv[:, t * 9 * 128 : (t + 1) * 9 * 128])

            # pre-warm the ACT function tables (1.3us each if loaded lazily
            # inside the latency-critical chains)
            actw = const.tile([128, 1], F32)
            zcol = zf[:, 0:1]
            nc.scalar.activation(actw[:], zcol, AF.Identity, bias=zcol)
            nc.scalar.activation(actw[:], zcol, AF.Relu, bias=zcol)
            nc.scalar.activation(actw[:], zcol, AF.Exp, bias=zcol)
            nc.scalar.copy(actw[:], zcol)

            # padded image built on-chip from slice 0 (a strided DMA here
            # would shatter into tiny descriptors and swamp the queues)
            xp = const.tile([128, WP * WP], BF16)
            xp3 = xp[:].rearrange("p (h w) -> p h w", w=WP)
            x03 = xa[:, 0:HW2].rearrange("p (h w) -> p h w", w=HW)
            nc.vector.tensor_copy(xp3[:, 1 : 1 + HW, 1 : 1 + HW], x03[:, :, :])
            nc.vector.tensor_copy(xp3[:, 0, 0:WP], zb[:, 0:WP])
            nc.vector.tensor_copy(xp3[:, WP - 1, 0:WP], zb[:, 0:WP])
            nc.vector.tensor_copy(xp3[:, 1 : 1 + HW, 0], zb[:, 0:HW])
            nc.vector.tensor_copy(xp3[:, 1 : 1 + HW, WP - 1], zb[:, 0:HW])

            # ---- global-average pooling of all 8 samples (DVE), emitted
            # chunk-by-chunk so each reduce starts as its slices land;
            # 1/HW2 is folded into w1 on the host ----
            pooled = const.tile([128, B], F32)
            xav = xa[:].rearrange("p (s f) -> p s f", s=B)
            nc.vector.tensor_reduce(
                pooled[:, 0:1], xav[:, 0:1, :], axis=AX.X, op=ALU.add
            )
            nc.vector.tensor_reduce(
                pooled[:, 1:4], xav[:, 1:4, :], axis=AX.X, op=ALU.add
            )
            nc.vector.tensor_reduce(
                pooled[:, 4:8], xav[:, 4:8, :], axis=AX.X, op=ALU.add
            )
            pooled8 = const.tile([128, B], BF16)
            nc.vector.tensor_copy(pooled8[:], pooled[:])

            cs_tiles = [None] * 8

            def emit_conv(t):
                cs = csb_pool.tile([128, HW2], BF16, tag="csb")
                cs_tiles[t] = cs
                for (r0, R) in ROW_GROUPS:
                    pt = psA.tile([128, R * HW], F32, tag="cps")
                    for tap in range(9):
                        dh, dw = divmod(tap, 3)
                        rhs = xp3[:, r0 + dh : r0 + dh + R, dw : dw + HW]
                        nc.tensor.matmul(
                            pt[:],
                            lhsT=wt[t][:, tap * 128 : (tap + 1) * 128],
                            rhs=rhs,
                            start=(tap == 0),
                            stop=(tap == 8),
                        )
                    # PSUM -> SBUF eviction, fused with the conv bias add
                    nc.scalar.activation(
                        cs[:, r0 * HW : (r0 + R) * HW],
                        pt[:],
                        AF.Identity,
                        bias=bct[:, t : t + 1],
                    )

            def emit_blend(t):
                cs = cs_tiles[t]
                for u in range(2):
                    g = 2 * t + u
                    ob = osb_pool.tile([128, HW2], F32, tag="osb")
                    for ci_, (c0, C) in enumerate(CHUNKS):
                        bp = psB.tile([128, C], F32, tag="bps")
                        nc.tensor.matmul(
                            bp[:],
                            lhsT=bd[:, 128 * u : 128 * u + 128],
                            rhs=cs[:, c0 : c0 + C],
                            start=True,
                            stop=True,
                        )
                        # PSUM drain balanced across DVE and ACT so psB bank
                        # recycling (not one engine) sets the blend rate
                        if ci_ in (1, 4):
                            nc.scalar.copy(ob[:, c0 : c0 + C], bp[:])
                        else:
                            nc.vector.tensor_copy(ob[:, c0 : c0 + C], bp[:])
                    nc.sync.dma_start(out[:, 16 * g : 16 * g + 16, :], ob[:])

            # conv t=0 first so the PE never waits on the attention pipeline
            emit_conv(0)

            # ---- attention MLP over all 8 pooled rows ----
            hd = []
            for h in range(2):
                hps = psM.tile([128, B], F32, tag="mlp")
                nc.tensor.matmul(
                    hps[:],
                    lhsT=w1s[:, h * 128 : (h + 1) * 128],
                    rhs=pooled8[:],
                    start=True,
                    stop=True,
                )
                hsb = const.tile([128, B], BF16, tag=f"hd{h}")
                nc.scalar.activation(hsb[:], hps[:], AF.Relu, bias=b1s[:, h : h + 1])
                hd.append(hsb)

            lps = psM.tile([B, K], F32, tag="mlp")
            nc.tensor.matmul(
                lps[:], lhsT=hd[0][:], rhs=w2s[:, 0:K], start=True, stop=False
            )
            nc.tensor.matmul(
                lps[:], lhsT=hd[1][:], rhs=w2s[:, K : 2 * K], start=False, stop=False
            )
            nc.tensor.matmul(
                lps[:], lhsT=ones[:], rhs=b2s[:], start=False, stop=True
            )

            # double softmax over k (shift-invariant: max-subtraction dropped)
            e1 = const.tile([B, K], F32)
            nc.scalar.activation(e1[:], lps[:], AF.Exp, bias=0.0, scale=1.0)
            s1 = const.tile([B, 1], F32)
            nc.vector.tensor_reduce(s1[:], e1[:], axis=AX.X, op=ALU.add)
            r1 = const.tile([B, 1], F32)
            nc.vector.reciprocal(r1[:], s1[:])
            a1 = const.tile([B, K], F32)
            nc.vector.tensor_scalar_mul(a1[:], e1[:], r1[:, 0:1])

            e2 = const.tile([B, K], F32)
            nc.scalar.activation(e2[:], a1[:], AF.Exp, bias=0.0, scale=1.0 / TAU)
            s2 = const.tile([B, 1], F32)
            nc.vector.tensor_reduce(s2[:], e2[:], axis=AX.X, op=ALU.add)
            r2 = const.tile([B, 1], F32)
            nc.vector.reciprocal(r2[:], s2[:])
            attn = const.tile([B, K], BF16)
            nc.vector.tensor_scalar_mul(attn[:], e2[:], r2[:, 0:1])

            emit_conv(1)

            # ---- blend lhsT: BD[p, u*128 + j*16 + c] = attn[j, k] at
            # partition p = 64u + 4c + k, zero elsewhere ----
            # attn [j, k] -> atT [k, j] via PE transpose
            tps = psM.tile([K, B], BF16, tag="mlp")
            nc.tensor.transpose(tps[:], attn[:], id8[:])
            atT = const.tile([K, B], BF16)
            nc.scalar.copy(atT[:], tps[:])
            # T1[p, j] = atT[p % 4, j]  (one-hot replicate matmul)
            t1p = psM.tile([128, B], F32, tag="mlp")
            nc.tensor.matmul(t1p[:], lhsT=k1s[:], rhs=atT[:], start=True, stop=True)
            t1 = const.tile([128, B], BF16)
            nc.scalar.copy(t1[:], t1p[:])
            # BD = T1 (broadcast over (u, c)) * CMASK
            bd = const.tile([128, 256], BF16)
            bdv = bd[:].rearrange("p (u j c) -> p u j c", u=2, c=16)
            cmv = cms[:].rearrange("p (u j c) -> p u j c", u=2, c=16)
            t1b = t1[:].rearrange("p (j c) -> p j c", c=1).broadcast_to([128, B, 16])
            for u in range(2):
                nc.vector.tensor_mul(bdv[:, u], cmv[:, u], t1b)

            # steady state: blend t chases conv t+1
            emit_blend(0)
            for t in range(2, 8):
                emit_conv(t)
                emit_blend(t - 1)
            emit_blend(7)

    nc.compile()
    return nc


def pack_inputs(x, conv_w, conv_b, w1, b1, w2, b2):
    """Host-side layout/dtype packing (no input-dependent arithmetic beyond
    constant folding of the mean-pool scale into w1)."""
    import ml_dtypes

    bf16 = ml_dtypes.bfloat16
    x = np.ascontiguousarray(x, dtype=np.float32)
    x_bf = x.reshape(B, CIN, HW2).astype(bf16)

    # conv_w [K, COUT, CIN, 3, 3] -> [ci, t, tap, p] with p = c*4 + k,
    # co = 32 t + c
    w = np.asarray(conv_w, dtype=np.float32).transpose(2, 3, 4, 0, 1)  # ci kh kw k co
    w = w.reshape(CIN, KS, KS, K, 8, 32)  # ci kh kw k t c
    w = w.transpose(0, 4, 1, 2, 5, 3)  # ci t kh kw c k
    wconv = np.ascontiguousarray(w.reshape(CIN, 8 * 9 * 128)).astype(bf16)

    bc = np.asarray(conv_b, dtype=np.float32).reshape(K, 8, 32)  # k t c
    bconv = np.ascontiguousarray(bc.transpose(1, 2, 0).reshape(8, 128).T)  # [p, t]

    w1t = (np.ascontiguousarray(np.asarray(w1, dtype=np.float32).T) / float(HW2)).astype(bf16)
    b1c = np.ascontiguousarray(np.asarray(b1, dtype=np.float32).reshape(2, 128).T)
    w2T = np.asarray(w2, dtype=np.float32).T  # [256, 4]
    w2t = np.ascontiguousarray(np.concatenate([w2T[:128], w2T[128:]], axis=1)).astype(bf16)
    b2r = np.asarray(b2, dtype=np.float32).reshape(1, K).astype(bf16)

    p = np.arange(128)
    col = np.arange(256)
    cmask = (
        ((col[None, :] // 128) == (p[:, None] // 64))
        & ((col[None, :] % 16) == ((p[:, None] % 64) // 4))
    ).astype(bf16)
    k1m = (np.arange(K)[:, None] == (p[None, :] % 4)).astype(bf16)

    common = dict(
        wconv=wconv, bconv=bconv, w1t=w1t, b1c=b1c, w2t=w2t, b2r=b2r,
        one18=np.ones((1, B), dtype=bf16),
        ident8=np.eye(B, dtype=bf16),
        k1m=k1m, cmask=cmask,
        zerb=np.zeros((128, 64), dtype=bf16),
        zerf=np.zeros((128, 8), dtype=np.float32),
    )
    in_maps = []
    for i in range(NCORES):
        # rotate so core i's own sample is slice 0
        xr = np.concatenate([x_bf[(i + r) % B] for r in range(B)], axis=1)
        in_maps.append(dict(common, xall=np.ascontiguousarray(xr)))
    return in_maps


def run(inputs, trace=False):
    from concourse.bass_utils import run_bass_kernel_spmd

    nc = build_nc()
    in_maps = pack_inputs(**inputs)
    res = run_bass_kernel_spmd(
        nc, in_maps, core_ids=list(range(NCORES)), trace=trace
    )
    out = np.empty((B, B, COUT, HW2), dtype=np.float32)
    for i in range(NCORES):
        slab = res.results[i]["out"]  # rows r correspond to j = (i + r) % B
        out[i] = slab[(np.arange(B) - i) % B]
    return out.reshape(B, B, COUT, HW, HW), res


def kernel(**inputs) -> np.ndarray:
    out, _ = run(inputs, trace=False)
    return out


# revision 6
# speedup vs baseline: 1.1704x; 1.0913x over previous
"""Trainium2 Bass kernel for nn_DynamicConv (dense_cnn).

out[i, j, co, h, w] = sum_k (conv_k(x_i)[co, h, w] + b_k[co]) * attn[j, k]
attn = softmax(softmax(MLP(meanpool(x)), k) / TAU, k)

Sharding: data-parallel over batch i across 8 cores.  All matmuls run in
bf16 (fp32r lowers to fp32_mode=HIGH which streams at 2 cycles/column —
half PE rate; bf16 streams 1 col/cycle, ~203ns per N=480 matmul with the
per-matmul LDWEIGHTS fully hidden by the PE reorder window).

No collective: every core receives ALL of x in bf16 (4.7 MB total, one
tile + DMA per sample so nothing serializes on whole-tile deps), rotated
so its own sample is sample 0, and computes the full [B, K] attention
matrix locally (per-sample global-average pools on DVE as slices land).
That removes the ~50us AllGather from the critical path entirely; the
host un-rotates the j rows of each core's output slab at gather time.

Conv: 9 shifted matmuls over a zero-padded image per 128-channel group
(contraction CIN=128).  Blend: one block-diagonal matmul per 16-channel
group (contraction 64 = (k=4) x (co16), M = 128 = (j=8) x (co16)); its
data-dependent lhsT (BD) is built on-chip in ~1.5us from attn via PE
transpose -> one-hot replicate matmul -> masked DVE multiply.  PE
instruction order is hand-arranged (conv0, MLP, conv1, BD, blend0,
conv2, blend1, ...) so the PE never waits on the attention pipeline.

Output-slab stores are issued as half-slabs rotated across the
sync/scalar HWDGE rings and the gpsimd SWDGE ring so the 18.9 MB output
write streams at HBM rate instead of one ring's and osb buffers recycle
quickly (a single ring was measured at ~200 GB/s and added a 35us tail).
"""

import sys

import numpy as np

if "/opt/trn_rl_repo" not in sys.path:
    sys.path.insert(0, "/opt/trn_rl_repo")

import concourse.bacc as bacc
import concourse.bass as bass
import concourse.mybir as mybir
import concourse.tile as tile

F32 = mybir.dt.float32
BF16 = mybir.dt.bfloat16
AF = mybir.ActivationFunctionType
AX = mybir.AxisListType
ALU = mybir.AluOpType

B = 8
CIN = 128
COUT = 256
K = 4
KS = 3
HW = 48
HW2 = HW * HW          # 2304
WP = HW + 2            # 50 (padded)
HID = 256
TAU = 30.0
NCORES = 8

ROW_GROUPS = [(0, 10), (10, 10), (20, 10), (30, 10), (40, 8)]
CHUNKS = [(0, 512), (512, 512), (1024, 512), (1536, 512), (2048, 256)]


def build_nc():
    nc = bacc.Bacc("TRN2", debug=False, num_devices=NCORES)

    # all 8 samples, rotated per-core so sample 0 is the core's own
    xall = nc.dram_tensor("xall", [CIN, B * HW2], BF16, kind="ExternalInput").ap()
    # [ci, t, tap, p] flattened; p = c*4 + k encodes (co = 32 t + c, k)
    wconv = nc.dram_tensor(
        "wconv", [CIN, 8 * 9 * 128], BF16, kind="ExternalInput"
    ).ap()
    bconv = nc.dram_tensor("bconv", [128, 8], F32, kind="ExternalInput").ap()
    w1t = nc.dram_tensor("w1t", [CIN, HID], BF16, kind="ExternalInput").ap()
    b1c = nc.dram_tensor("b1c", [128, 2], F32, kind="ExternalInput").ap()
    w2t = nc.dram_tensor("w2t", [128, 2 * K], BF16, kind="ExternalInput").ap()
    b2r = nc.dram_tensor("b2r", [1, K], BF16, kind="ExternalInput").ap()
    one18 = nc.dram_tensor("one18", [1, B], BF16, kind="ExternalInput").ap()
    ident8 = nc.dram_tensor("ident8", [B, B], BF16, kind="ExternalInput").ap()
    k1m = nc.dram_tensor("k1m", [K, 128], BF16, kind="ExternalInput").ap()
    cmask = nc.dram_tensor("cmask", [128, 256], BF16, kind="ExternalInput").ap()
    zerb = nc.dram_tensor("zerb", [128, 64], BF16, kind="ExternalInput").ap()
    zerf = nc.dram_tensor("zerf", [128, 8], F32, kind="ExternalInput").ap()
    out = nc.dram_tensor("out", [B, COUT, HW2], F32, kind="ExternalOutput").ap()

    with tile.TileContext(nc, num_cores=NCORES) as tc:
        with (
            tc.tile_pool(name="const", bufs=1) as const,
            tc.tile_pool(name="csb", bufs=3) as csb_pool,
            tc.tile_pool(name="osb", bufs=5) as osb_pool,
            tc.tile_pool(name="psA", bufs=3, space="PSUM") as psA,
            tc.tile_pool(name="psB", bufs=4, space="PSUM") as psB,
            tc.tile_pool(name="psM", bufs=1, space="PSUM") as psM,
        ):
            # ---- DMA issue; one tile per sample so dependencies stay
            # per-sample (Tile tracks deps at tile granularity) ----
            zb = const.tile([128, 64], BF16)
            nc.sync.dma_start(zb[:], zerb[:, :])
            zf = const.tile([128, 8], F32)
            nc.sync.dma_start(zf[:], zerf[:, :])
            xs = []
            for s in range(B):
                xt = const.tile([128, HW2], BF16, tag=f"xs{s}")
                xs.append(xt)
            nc.sync.dma_start(xs[0][:], xall[:, 0:HW2])
            wt = []
            for t in range(8):
                w = const.tile([128, 9 * 128], BF16, tag=f"wt{t}")
                wt.append(w)
            nc.sync.dma_start(wt[0][:], wconv[:, 0 : 9 * 128])
            nc.sync.dma_start(wt[1][:], wconv[:, 9 * 128 : 2 * 9 * 128])
            for s in range(1, 4):
                nc.scalar.dma_start(xs[s][:], xall[:, s * HW2 : (s + 1) * HW2])
            for s in range(4, 8):
                nc.gpsimd.dma_start(xs[s][:], xall[:, s * HW2 : (s + 1) * HW2])
            # small constants
            bct = const.tile([128, 8], F32)
            nc.scalar.dma_start(bct[:], bconv[:, :])
            w1s = const.tile([128, HID], BF16)
            nc.scalar.dma_start(w1s[:], w1t[:, :])
            b1s = const.tile([128, 2], F32)
            nc.scalar.dma_start(b1s[:], b1c[:, :])
            w2s = const.tile([128, 2 * K], BF16)
            nc.scalar.dma_start(w2s[:], w2t[:, :])
            b2s = const.tile([1, K], BF16)
            nc.scalar.dma_start(b2s[:], b2r[:, :])
            ones = const.tile([1, B], BF16)
            nc.scalar.dma_start(ones[:], one18[:, :])
            id8 = const.tile([B, B], BF16)
            nc.scalar.dma_start(id8[:], ident8[:, :])
            k1s = const.tile([K, 128], BF16)
            nc.scalar.dma_start(k1s[:], k1m[:, :])
            cms = const.tile([128, 256], BF16)
            nc.scalar.dma_start(cms[:], cmask[:, :])
            # remaining conv weights (needed one per ~9us of conv)
            for t in range(2, 8):
                eng = nc.scalar if t % 2 == 0 else nc.gpsimd
                eng.dma_start(wt[t][:], wconv[:, t * 9 * 128 : (t + 1) * 9 * 128])

            # pre-warm the ACT function tables
            actw = const.tile([128, 1], F32)
            zcol = zf[:, 0:1]
            nc.scalar.activation(actw[:], zcol, AF.Identity, bias=zcol)
            nc.scalar.activation(actw[:], zcol, AF.Relu, bias=zcol)
            nc.scalar.activation(actw[:], zcol, AF.Exp, bias=zcol)
            nc.scalar.copy(actw[:], zcol)

            # padded image built on-chip from the core's own sample
            xp = const.tile([128, WP * WP], BF16)
            xp3 = xp[:].rearrange("p (h w) -> p h w", w=WP)
            x03 = xs[0][:].rearrange("p (h w) -> p h w", w=HW)
            nc.vector.tensor_copy(xp3[:, 1 : 1 + HW, 1 : 1 + HW], x03[:, :, :])
            nc.vector.tensor_copy(xp3[:, 0, 0:WP], zb[:, 0:WP])
            nc.vector.tensor_copy(xp3[:, WP - 1, 0:WP], zb[:, 0:WP])
            nc.vector.tensor_copy(xp3[:, 1 : 1 + HW, 0], zb[:, 0:HW])
            nc.vector.tensor_copy(xp3[:, 1 : 1 + HW, WP - 1], zb[:, 0:HW])

            # per-sample global-average pools (DVE), pipelined with arrival
            pooled = const.tile([128, B], F32)
            for s in range(B):
                nc.vector.tensor_reduce(
                    pooled[:, s : s + 1], xs[s][:], axis=AX.X, op=ALU.add
                )
            pooled8 = const.tile([128, B], BF16)
            nc.vector.tensor_copy(pooled8[:], pooled[:])

            cs_tiles = [None] * 8

            def emit_conv(t):
                cs = csb_pool.tile([128, HW2], BF16, tag="csb")
                cs_tiles[t] = cs
                for (r0, R) in ROW_GROUPS:
                    pt = psA.tile([128, R * HW], F32, tag="cps")
                    for tap in range(9):
                        dh, dw = divmod(tap, 3)
                        rhs = xp3[:, r0 + dh : r0 + dh + R, dw : dw + HW]
                        nc.tensor.matmul(
                            pt[:],
                            lhsT=wt[t][:, tap * 128 : (tap + 1) * 128],
                            rhs=rhs,
                            start=(tap == 0),
                            stop=(tap == 8),
                        )
                    # PSUM -> SBUF eviction, fused with the conv bias add
                    nc.scalar.activation(
                        cs[:, r0 * HW : (r0 + R) * HW],
                        pt[:],
                        AF.Identity,
                        bias=bct[:, t : t + 1],
                    )

            dma_rr = [0]

            def emit_blend(t):
                cs = cs_tiles[t]
                for u in range(2):
                    g = 2 * t + u
                    ob = osb_pool.tile([128, HW2], F32, tag="osb")
                    for ci_, (c0, C) in enumerate(CHUNKS):
                        bp = psB.tile([128, C], F32, tag="bps")
                        nc.tensor.matmul(
                            bp[:],
                            lhsT=bd[:, 128 * u : 128 * u + 128],
                            rhs=cs[:, c0 : c0 + C],
                            start=True,
                            stop=True,
                        )
                        # PSUM drain balanced across DVE and ACT so psB bank
                        # recycling (not one engine) sets the blend rate
                        if ci_ in (1, 4):
                            nc.scalar.copy(ob[:, c0 : c0 + C], bp[:])
                        else:
                            nc.vector.tensor_copy(ob[:, c0 : c0 + C], bp[:])
                    # half-slab stores rotated across three DMA rings
                    for h0, H in ((0, HW2 // 2), (HW2 // 2, HW2 // 2)):
                        eng = (nc.sync, nc.scalar, nc.gpsimd)[dma_rr[0] % 3]
                        dma_rr[0] += 1
                        eng.dma_start(
                            out[:, 16 * g : 16 * g + 16, h0 : h0 + H],
                            ob[:, h0 : h0 + H],
                        )

            # conv t=0 first so the PE never waits on the attention pipeline
            emit_conv(0)

            # ---- attention MLP over all 8 pooled rows ----
            hd = []
            for h in range(2):
                hps = psM.tile([128, B], F32, tag="mlp")
                nc.tensor.matmul(
                    hps[:],
                    lhsT=w1s[:, h * 128 : (h + 1) * 128],
                    rhs=pooled8[:],
                    start=True,
                    stop=True,
                )
                hsb = const.tile([128, B], BF16, tag=f"hd{h}")
                nc.scalar.activation(hsb[:], hps[:], AF.Relu, bias=b1s[:, h : h + 1])
                hd.append(hsb)

            lps = psM.tile([B, K], F32, tag="mlp")
            nc.tensor.matmul(
                lps[:], lhsT=hd[0][:], rhs=w2s[:, 0:K], start=True, stop=False
            )
            nc.tensor.matmul(
                lps[:], lhsT=hd[1][:], rhs=w2s[:, K : 2 * K], start=False, stop=False
            )
            nc.tensor.matmul(
                lps[:], lhsT=ones[:], rhs=b2s[:], start=False, stop=True
            )

            # double softmax over k (shift-invariant: max-subtraction dropped)
            e1 = const.tile([B, K], F32)
            nc.scalar.activation(e1[:], lps[:], AF.Exp, bias=0.0, scale=1.0)
            s1 = const.tile([B, 1], F32)
            nc.vector.tensor_reduce(s1[:], e1[:], axis=AX.X, op=ALU.add)
            r1 = const.tile([B, 1], F32)
            nc.vector.reciprocal(r1[:], s1[:])
            a1 = const.tile([B, K], F32)
            nc.vector.tensor_scalar_mul(a1[:], e1[:], r1[:, 0:1])

            e2 = const.tile([B, K], F32)
            nc.scalar.activation(e2[:], a1[:], AF.Exp, bias=0.0, scale=1.0 / TAU)
            s2 = const.tile([B, 1], F32)
            nc.vector.tensor_reduce(s2[:], e2[:], axis=AX.X, op=ALU.add)
            r2 = const.tile([B, 1], F32)
            nc.vector.reciprocal(r2[:], s2[:])
            attn = const.tile([B, K], BF16)
            nc.vector.tensor_scalar_mul(attn[:], e2[:], r2[:, 0:1])

            emit_conv(1)

            # ---- blend lhsT: BD[p, u*128 + j*16 + c] = attn[j, k] at
            # partition p = 64u + 4c + k, zero elsewhere ----
            tps = psM.tile([K, B], BF16, tag="mlp")
            nc.tensor.transpose(tps[:], attn[:], id8[:])
            atT = const.tile([K, B], BF16)
            nc.scalar.copy(atT[:], tps[:])
            # T1[p, j] = atT[p % 4, j]  (one-hot replicate matmul)
            t1p = psM.tile([128, B], F32, tag="mlp")
            nc.tensor.matmul(t1p[:], lhsT=k1s[:], rhs=atT[:], start=True, stop=True)
            t1 = const.tile([128, B], BF16)
            nc.scalar.copy(t1[:], t1p[:])
            # BD = T1 (broadcast over (u, c)) * CMASK
            bd = const.tile([128, 256], BF16)
            bdv = bd[:].rearrange("p (u j c) -> p u j c", u=2, c=16)
            cmv = cms[:].rearrange("p (u j c) -> p u j c", u=2, c=16)
            t1b = t1[:].rearrange("p (j c) -> p j c", c=1).broadcast_to([128, B, 16])
            for u in range(2):
                nc.vector.tensor_mul(bdv[:, u], cmv[:, u], t1b)

            # steady state: blend t chases conv t+1
            emit_blend(0)
            for t in range(2, 8):
                emit_conv(t)
                emit_blend(t - 1)
            emit_blend(7)

    nc.compile()
    return nc


def pack_inputs(x, conv_w, conv_b, w1, b1, w2, b2):
    """Host-side layout/dtype packing (no input-dependent arithmetic beyond
    constant folding of the mean-pool scale into w1)."""
    import ml_dtypes

    bf16 = ml_dtypes.bfloat16
    x = np.ascontiguousarray(x, dtype=np.float32)
    x_bf = x.reshape(B, CIN, HW2).astype(bf16)

    # conv_w [K, COUT, CIN, 3, 3] -> [ci, t, tap, p] with p = c*4 + k,
    # co = 32 t + c
    w = np.asarray(conv_w, dtype=np.float32).transpose(2, 3, 4, 0, 1)  # ci kh kw k co
    w = w.reshape(CIN, KS, KS, K, 8, 32)  # ci kh kw k t c
    w = w.transpose(0, 4, 1, 2, 5, 3)  # ci t kh kw c k
    wconv = np.ascontiguousarray(w.reshape(CIN, 8 * 9 * 128)).astype(bf16)

    bc = np.asarray(conv_b, dtype=np.float32).reshape(K, 8, 32)  # k t c
    bconv = np.ascontiguousarray(bc.transpose(1, 2, 0).reshape(8, 128).T)  # [p, t]

    w1t = (np.ascontiguousarray(np.asarray(w1, dtype=np.float32).T) / float(HW2)).astype(bf16)
    b1c = np.ascontiguousarray(np.asarray(b1, dtype=np.float32).reshape(2, 128).T)
    w2T = np.asarray(w2, dtype=np.float32).T  # [256, 4]
    w2t = np.ascontiguousarray(np.concatenate([w2T[:128], w2T[128:]], axis=1)).astype(bf16)
    b2r = np.asarray(b2, dtype=np.float32).reshape(1, K).astype(bf16)

    p = np.arange(128)
    col = np.arange(256)
    cmask = (
        ((col[None, :] // 128) == (p[:, None] // 64))
        & ((col[None, :] % 16) == ((p[:, None] % 64) // 4))
    ).astype(bf16)
    k1m = (np.arange(K)[:, None] == (p[None, :] % 4)).astype(bf16)

    common = dict(
        wconv=wconv, bconv=bconv, w1t=w1t, b1c=b1c, w2t=w2t, b2r=b2r,
        one18=np.ones((1, B), dtype=bf16),
        ident8=np.eye(B, dtype=bf16),
        k1m=k1m, cmask=cmask,
        zerb=np.zeros((128, 64), dtype=bf16),
        zerf=np.zeros((128, 8), dtype=np.float32),
    )
    in_maps = []
    for i in range(NCORES):
        # rotate so core i's own sample is slice 0
        xr = np.concatenate([x_bf[(i + r) % B] for r in range(B)], axis=1)
        in_maps.append(dict(common, xall=np.ascontiguousarray(xr)))
    return in_maps


def run(inputs, trace=False):
    from concourse.bass_utils import run_bass_kernel_spmd

    nc = build_nc()
    in_maps = pack_inputs(**inputs)
    res = run_bass_kernel_spmd(
        nc, in_maps, core_ids=list(range(NCORES)), trace=trace
    )
    out = np.empty((B, B, COUT, HW2), dtype=np.float32)
    for i in range(NCORES):
        slab = res.results[i]["out"]  # rows r correspond to j = (i + r) % B
        out[i] = slab[(np.arange(B) - i) % B]
    return out.reshape(B, B, COUT, HW, HW), res


def kernel(**inputs) -> np.ndarray:
    out, _ = run(inputs, trace=False)
    return out


# revision 8
# speedup vs baseline: 1.3400x; 1.1450x over previous
"""Trainium2 Bass kernel for nn_DynamicConv (dense_cnn).

out[i, j, co, h, w] = sum_k (conv_k(x_i)[co, h, w] + b_k[co]) * attn[j, k]
attn = softmax(softmax(MLP(meanpool(x)), k) / TAU, k)

Sharding: data-parallel over batch i across 8 cores.  All matmuls run in
bf16 (fp32r lowers to fp32_mode=HIGH at 2 cycles/column — half PE rate;
bf16 streams 1 col/cycle, ~203ns per N=480 matmul with per-MM LDWEIGHTS
fully hidden).

No collective: every core receives ALL of x in bf16, rotated so its own
sample is sample 0, and computes the full [B, K] attention matrix
locally; the host un-rotates the j rows of each core's output slab at
gather time.  (The AllGather path costs ~50us wall for 16 bytes.)

DMA layout rules learned from traces (per-queue throughput is
descriptor-bound at ~40ns/descriptor, so line length rules):
  - samples ship solo (s0, s1) or column-paired (4608B -> 9216B lines),
  - conv weights ship as two quad-t blocks (9216B lines),
  - every DRAM region touched by one dma_start is fully contiguous,
  - all small constants ride in ONE 128x680 bf16 blob (f32 views via
    bitcast), since each tiny transfer costs 128 descriptors (~2-3us of
    queue time) — two of them in front of x_0 cost 6us of startup,
  - the output is group-major [g, j, c, hw] so each blend group's slab
    store is one contiguous 1.18MB write (host transposes at gather);
    slab stores rotate across the sync/scalar HWDGE + gpsimd SWDGE rings.

PE order is hand-arranged (c0, c1, MLP+softmax, c2, BD, b0, c3, b1, b2,
c4, b3, c5, b4, c6, b5, b6, c7, b7) so the PE never waits on the
attention pipeline and only one blend remains after the last conv.
"""

import sys

import numpy as np

if "/opt/trn_rl_repo" not in sys.path:
    sys.path.insert(0, "/opt/trn_rl_repo")

import concourse.bacc as bacc
import concourse.bass as bass
import concourse.mybir as mybir
import concourse.tile as tile

F32 = mybir.dt.float32
BF16 = mybir.dt.bfloat16
AF = mybir.ActivationFunctionType
AX = mybir.AxisListType
ALU = mybir.AluOpType

B = 8
CIN = 128
COUT = 256
K = 4
KS = 3
HW = 48
HW2 = HW * HW          # 2304
WP = HW + 2            # 50 (padded)
HID = 256
TAU = 30.0
NCORES = 8

ROW_GROUPS = [(0, 10), (10, 10), (20, 10), (30, 10), (40, 8)]
CHUNKS = [(0, 512), (512, 512), (1024, 512), (1536, 512), (2048, 256)]

# const blob column map (bf16 units)
CB_W1 = 0          # [128, 256] bf16
CB_CM = 256        # [128, 256] bf16 cmask
CB_BCT = 512       # [128, 16] bf16 view = [128, 8] f32 conv bias
CB_B1 = 528        # [128, 4] bf16 view = [128, 2] f32 mlp bias1
CB_K1 = 532        # [4, 128] bf16 one-hot replicate
CB_ID8 = 660       # [8, 8] bf16 identity
CB_B2 = 668        # [1, 4] bf16 mlp bias2
CB_ONE = 672       # [1, 8] bf16 ones
CB_W2 = 680        # [128, 8] bf16 w2 (two 128-col halves stacked)
CB_COLS = 688


def build_nc():
    nc = bacc.Bacc("TRN2", debug=False, num_devices=NCORES)

    # samples 0,1 solo (rows (s, ci)); samples 2..7 column-paired
    xsolo = nc.dram_tensor("xsolo", [2 * CIN, HW2], BF16, kind="ExternalInput").ap()
    xpack = nc.dram_tensor(
        "xpack", [3 * CIN, 2 * HW2], BF16, kind="ExternalInput"
    ).ap()
    # conv weights as four pair-t blocks: rows (q, ci), line (tq, tap, p)
    wconv2 = nc.dram_tensor(
        "wconv2", [4 * CIN, 2 * 9 * 128], BF16, kind="ExternalInput"
    ).ap()
    cblob = nc.dram_tensor("cblob", [128, CB_COLS], BF16, kind="ExternalInput").ap()
    # group-major output: [g, j, c, hw] — each [g] slab is contiguous
    out = nc.dram_tensor("out", [16, B, 16, HW2], F32, kind="ExternalOutput").ap()

    with tile.TileContext(nc, num_cores=NCORES) as tc:
        with (
            tc.tile_pool(name="const", bufs=1) as const,
            tc.tile_pool(name="csb", bufs=4) as csb_pool,
            tc.tile_pool(name="osb", bufs=5) as osb_pool,
            tc.tile_pool(name="psA", bufs=3, space="PSUM") as psA,
            tc.tile_pool(name="psB", bufs=4, space="PSUM") as psB,
            tc.tile_pool(name="psM", bufs=1, space="PSUM") as psM,
        ):
            # ---- DMA issue (sync: own sample + first weights; scalar:
            # consts + pairs; gpsimd: pair + second weights) ----
            wtq = []
            for q in range(4):
                w = const.tile([128, 2 * 9 * 128], BF16, tag=f"wtq{q}")
                wtq.append(w)
            xs0 = const.tile([128, HW2], BF16)
            nc.sync.dma_start(xs0[:], xsolo[0:128, :])
            xs1 = const.tile([128, HW2], BF16)
            nc.sync.dma_start(xs1[:], xsolo[128:256, :])
            nc.sync.dma_start(wtq[1][:], wconv2[128:256, :])

            nc.scalar.dma_start(wtq[0][:], wconv2[0:128, :])
            blob = const.tile([128, CB_COLS], BF16)
            nc.scalar.dma_start(blob[:], cblob[:, :])
            xp23 = const.tile([128, 2 * HW2], BF16)
            nc.scalar.dma_start(xp23[:], xpack[0:128, :])
            xp67 = const.tile([128, 2 * HW2], BF16)
            nc.scalar.dma_start(xp67[:], xpack[256:384, :])

            xp45 = const.tile([128, 2 * HW2], BF16)
            nc.gpsimd.dma_start(xp45[:], xpack[128:256, :])
            nc.gpsimd.dma_start(wtq[2][:], wconv2[256:384, :])
            nc.gpsimd.dma_start(wtq[3][:], wconv2[384:512, :])

            # const views out of the blob
            w1s = blob[:, CB_W1 : CB_W1 + 256]
            cms = blob[:, CB_CM : CB_CM + 256]
            bct = blob[:, CB_BCT : CB_BCT + 16].bitcast(F32)
            b1s = blob[:, CB_B1 : CB_B1 + 4].bitcast(F32)
            k1s = blob[0:K, CB_K1 : CB_K1 + 128]
            id8 = blob[0:B, CB_ID8 : CB_ID8 + 8]
            b2s = blob[0:1, CB_B2 : CB_B2 + K]
            ones = blob[0:1, CB_ONE : CB_ONE + B]
            w2s = blob[:, CB_W2 : CB_W2 + 2 * K]

            # ACT table pre-warm off a memset zero column
            zf = const.tile([128, 1], F32)
            nc.gpsimd.memset(zf[:], 0.0)
            actw = const.tile([128, 1], F32)
            zcol = zf[:, 0:1]
            nc.scalar.activation(actw[:], zcol, AF.Identity, bias=zcol)
            nc.scalar.activation(actw[:], zcol, AF.Relu, bias=zcol)
            nc.scalar.activation(actw[:], zcol, AF.Exp, bias=zcol)
            nc.scalar.copy(actw[:], zcol)

            # padded image: borders memset (gpsimd), interior copied (DVE)
            xp = const.tile([128, WP * WP], BF16)
            xp3 = xp[:].rearrange("p (h w) -> p h w", w=WP)
            nc.gpsimd.memset(xp3[:, 0, 0:WP], 0.0)
            nc.gpsimd.memset(xp3[:, WP - 1, 0:WP], 0.0)
            nc.gpsimd.memset(xp3[:, 1 : 1 + HW, 0], 0.0)
            nc.gpsimd.memset(xp3[:, 1 : 1 + HW, WP - 1], 0.0)
            x03 = xs0[:].rearrange("p (h w) -> p h w", w=HW)
            nc.vector.tensor_copy(xp3[:, 1 : 1 + HW, 1 : 1 + HW], x03[:, :, :])

            # global-average pools (DVE), pipelined with arrival;
            # 1/HW2 is folded into w1 on the host
            pooled = const.tile([128, B], F32)
            nc.vector.tensor_reduce(pooled[:, 0:1], xs0[:], axis=AX.X, op=ALU.add)
            nc.vector.tensor_reduce(pooled[:, 1:2], xs1[:], axis=AX.X, op=ALU.add)
            for s0_, tl in ((2, xp23), (4, xp45), (6, xp67)):
                v = tl[:].rearrange("p (s f) -> p s f", s=2)
                nc.vector.tensor_reduce(
                    pooled[:, s0_ : s0_ + 2], v, axis=AX.X, op=ALU.add
                )
            pooled8 = const.tile([128, B], BF16)
            nc.vector.tensor_copy(pooled8[:], pooled[:])

            cs_tiles = [None] * 8

            def emit_conv(t):
                cs = csb_pool.tile([128, HW2], BF16, tag="csb")
                cs_tiles[t] = cs
                wq = wtq[t // 2]
                base = (t % 2) * 9 * 128
                for (r0, R) in ROW_GROUPS:
                    pt = psA.tile([128, R * HW], F32, tag="cps")
                    for tap in range(9):
                        dh, dw = divmod(tap, 3)
                        rhs = xp3[:, r0 + dh : r0 + dh + R, dw : dw + HW]
                        nc.tensor.matmul(
                            pt[:],
                            lhsT=wq[:, base + tap * 128 : base + (tap + 1) * 128],
                            rhs=rhs,
                            start=(tap == 0),
                            stop=(tap == 8),
                        )
                    # PSUM -> SBUF eviction, fused with the conv bias add
                    nc.scalar.activation(
                        cs[:, r0 * HW : (r0 + R) * HW],
                        pt[:],
                        AF.Identity,
                        bias=bct[:, t : t + 1],
                    )

            dma_rr = [0]

            def emit_blend(t):
                cs = cs_tiles[t]
                for u in range(2):
                    g = 2 * t + u
                    ob = osb_pool.tile([128, HW2], F32, tag="osb")
                    for ci_, (c0, C) in enumerate(CHUNKS):
                        bp = psB.tile([128, C], F32, tag="bps")
                        nc.tensor.matmul(
                            bp[:],
                            lhsT=bd[:, 128 * u : 128 * u + 128],
                            rhs=cs[:, c0 : c0 + C],
                            start=True,
                            stop=True,
                        )
                        # PSUM drain balanced across DVE and ACT so psB bank
                        # recycling (not one engine) sets the blend rate
                        if ci_ in (1, 4):
                            nc.scalar.copy(ob[:, c0 : c0 + C], bp[:])
                        else:
                            nc.vector.tensor_copy(ob[:, c0 : c0 + C], bp[:])
                    # one contiguous full-slab store, rings rotated
                    eng = (nc.sync, nc.scalar, nc.gpsimd)[dma_rr[0] % 3]
                    dma_rr[0] += 1
                    eng.dma_start(out[g, :, :, :], ob[:])

            emit_conv(0)
            emit_conv(1)

            # ---- attention MLP over all 8 pooled rows ----
            hd = []
            for h in range(2):
                hps = psM.tile([128, B], F32, tag="mlp")
                nc.tensor.matmul(
                    hps[:],
                    lhsT=w1s[:, h * 128 : (h + 1) * 128],
                    rhs=pooled8[:],
                    start=True,
                    stop=True,
                )
                hsb = const.tile([128, B], BF16, tag=f"hd{h}")
                nc.scalar.activation(hsb[:], hps[:], AF.Relu, bias=b1s[:, h : h + 1])
                hd.append(hsb)

            lps = psM.tile([B, K], F32, tag="mlp")
            nc.tensor.matmul(
                lps[:], lhsT=hd[0][:], rhs=w2s[:, 0:K], start=True, stop=False
            )
            nc.tensor.matmul(
                lps[:], lhsT=hd[1][:], rhs=w2s[:, K : 2 * K], start=False, stop=False
            )
            nc.tensor.matmul(
                lps[:], lhsT=ones, rhs=b2s, start=False, stop=True
            )

            # double softmax over k (shift-invariant: max-subtraction dropped)
            e1 = const.tile([B, K], F32)
            nc.scalar.activation(e1[:], lps[:], AF.Exp, bias=0.0, scale=1.0)
            s1 = const.tile([B, 1], F32)
            nc.vector.tensor_reduce(s1[:], e1[:], axis=AX.X, op=ALU.add)
            r1 = const.tile([B, 1], F32)
            nc.vector.reciprocal(r1[:], s1[:])
            a1 = const.tile([B, K], F32)
            nc.vector.tensor_scalar_mul(a1[:], e1[:], r1[:, 0:1])

            e2 = const.tile([B, K], F32)
            nc.scalar.activation(e2[:], a1[:], AF.Exp, bias=0.0, scale=1.0 / TAU)
            s2 = const.tile([B, 1], F32)
            nc.vector.tensor_reduce(s2[:], e2[:], axis=AX.X, op=ALU.add)
            r2 = const.tile([B, 1], F32)
            nc.vector.reciprocal(r2[:], s2[:])
            attn = const.tile([B, K], BF16)
            nc.vector.tensor_scalar_mul(attn[:], e2[:], r2[:, 0:1])

            emit_conv(2)

            # ---- blend lhsT: BD[p, u*128 + j*16 + c] = attn[j, k] at
            # partition p = 64u + 4c + k, zero elsewhere ----
            tps = psM.tile([K, B], BF16, tag="mlp")
            nc.tensor.transpose(tps[:], attn[:], id8)
            atT = const.tile([K, B], BF16)
            nc.scalar.copy(atT[:], tps[:])
            # T1[p, j] = atT[p % 4, j]  (one-hot replicate matmul)
            t1p = psM.tile([128, B], F32, tag="mlp")
            nc.tensor.matmul(t1p[:], lhsT=k1s, rhs=atT[:], start=True, stop=True)
            t1 = const.tile([128, B], BF16)
            nc.scalar.copy(t1[:], t1p[:])
            # BD = T1 (broadcast over (u, c)) * CMASK
            bd = const.tile([128, 256], BF16)
            bdv = bd[:].rearrange("p (u j c) -> p u j c", u=2, c=16)
            cmv = cms.rearrange("p (u j c) -> p u j c", u=2, c=16)
            t1b = t1[:].rearrange("p (j c) -> p j c", c=1).broadcast_to([128, B, 16])
            for u in range(2):
                nc.vector.tensor_mul(bdv[:, u], cmv[:, u], t1b)

            # steady state; catch blends up so only b7 trails conv7
            emit_blend(0)
            emit_conv(3)
            emit_blend(1)
            emit_blend(2)
            emit_conv(4)
            emit_blend(3)
            emit_conv(5)
            emit_blend(4)
            emit_conv(6)
            emit_blend(5)
            emit_blend(6)
            emit_conv(7)
            emit_blend(7)

    nc.compile()
    return nc


def pack_inputs(x, conv_w, conv_b, w1, b1, w2, b2):
    """Host-side layout/dtype packing (no input-dependent arithmetic beyond
    constant folding of the mean-pool scale into w1)."""
    import ml_dtypes

    bf16 = ml_dtypes.bfloat16
    x = np.ascontiguousarray(x, dtype=np.float32)
    x_bf = x.reshape(B, CIN, HW2).astype(bf16)

    # conv_w [K, COUT, CIN, 3, 3] -> [ci, t, tap, p] with p = c*4 + k,
    # co = 32 t + c; then regrouped into two quad-t blocks (q, ci, tq, tap, p)
    w = np.asarray(conv_w, dtype=np.float32).transpose(2, 3, 4, 0, 1)  # ci kh kw k co
    w = w.reshape(CIN, KS, KS, K, 8, 32)  # ci kh kw k t c
    w = w.transpose(0, 4, 1, 2, 5, 3)  # ci t kh kw c k
    wfull = w.reshape(CIN, 8, 9 * 128)  # ci t (tap p)
    wconv2 = np.ascontiguousarray(
        wfull.reshape(CIN, 4, 2, 9 * 128).transpose(1, 0, 2, 3).reshape(
            4 * CIN, 2 * 9 * 128
        )
    ).astype(bf16)

    bc = np.asarray(conv_b, dtype=np.float32).reshape(K, 8, 32)  # k t c
    bconv = np.ascontiguousarray(bc.transpose(1, 2, 0).reshape(8, 128).T)  # [p, t]

    w1t = (np.ascontiguousarray(np.asarray(w1, dtype=np.float32).T) / float(HW2)).astype(bf16)
    b1c = np.ascontiguousarray(np.asarray(b1, dtype=np.float32).reshape(2, 128).T)
    w2T = np.asarray(w2, dtype=np.float32).T  # [256, 4]
    w2t = np.ascontiguousarray(np.concatenate([w2T[:128], w2T[128:]], axis=1)).astype(bf16)

    p = np.arange(128)
    col = np.arange(256)
    cmask = (
        ((col[None, :] // 128) == (p[:, None] // 64))
        & ((col[None, :] % 16) == ((p[:, None] % 64) // 4))
    ).astype(bf16)
    k1m = (np.arange(K)[:, None] == (p[None, :] % 4)).astype(bf16)

    # single constant blob [128, CB_COLS] bf16 (f32 values bitcast into pairs)
    cb = np.zeros((128, CB_COLS), dtype=bf16)
    cb[:, CB_W1 : CB_W1 + 256] = w1t
    cb[:, CB_CM : CB_CM + 256] = cmask
    cb[:, CB_BCT : CB_BCT + 16] = bconv.astype(np.float32).view(bf16)[:, : 16]
    cb[:, CB_B1 : CB_B1 + 4] = b1c.astype(np.float32).view(bf16)[:, : 4]
    cb[0:K, CB_K1 : CB_K1 + 128] = k1m
    cb[0:B, CB_ID8 : CB_ID8 + 8] = np.eye(B, dtype=bf16)
    cb[0:1, CB_B2 : CB_B2 + K] = np.asarray(b2, dtype=np.float32).reshape(1, K).astype(bf16)
    cb[0:1, CB_ONE : CB_ONE + B] = np.ones((1, B), dtype=bf16)
    cb[:, CB_W2 : CB_W2 + 2 * K] = w2t

    common = dict(wconv2=wconv2, cblob=cb)
    in_maps = []
    for i in range(NCORES):
        # rotate so core i's own sample is slice 0
        xr = [x_bf[(i + r) % B] for r in range(B)]
        xsolo = np.ascontiguousarray(np.concatenate([xr[0], xr[1]], axis=0))
        xpack = np.ascontiguousarray(
            np.concatenate(
                [np.concatenate([xr[2 * q], xr[2 * q + 1]], axis=1) for q in (1, 2, 3)],
                axis=0,
            )
        )
        in_maps.append(dict(common, xsolo=xsolo, xpack=xpack))
    return in_maps


def run(inputs, trace=False):
    from concourse.bass_utils import run_bass_kernel_spmd

    nc = build_nc()
    in_maps = pack_inputs(**inputs)
    res = run_bass_kernel_spmd(
        nc, in_maps, core_ids=list(range(NCORES)), trace=trace
    )
    out = np.empty((B, B, COUT, HW2), dtype=np.float32)
    for i in range(NCORES):
        slab = res.results[i]["out"]  # [g, j(rot), c, hw]
        slab = slab.transpose(1, 0, 2, 3).reshape(B, COUT, HW2)
        out[i] = slab[(np.arange(B) - i) % B]
    return out.reshape(B, B, COUT, HW, HW), res


def kernel(**inputs) -> np.ndarray:
    out, _ = run(inputs, trace=False)
    return out


# revision 9
# speedup vs baseline: 1.3457x; 1.0042x over previous
"""Trainium2 Bass kernel for nn_DynamicConv (dense_cnn).

out[i, j, co, h, w] = sum_k (conv_k(x_i)[co, h, w] + b_k[co]) * attn[j, k]
attn = softmax(softmax(MLP(meanpool(x)), k) / TAU, k)

Sharding: data-parallel over batch i across 8 cores.  All matmuls run in
bf16 (fp32r lowers to fp32_mode=HIGH at 2 cycles/column — half PE rate;
bf16 streams 1 col/cycle, ~203ns per N=480 matmul with per-MM LDWEIGHTS
fully hidden).

No collective: every core receives ALL of x in bf16, rotated so its own
sample is sample 0, and computes the full [B, K] attention matrix
locally; the host un-rotates the j rows of each core's output slab at
gather time.  (The AllGather path costs ~50us wall for 16 bytes.)

DMA layout rules learned from traces (per-queue throughput is
descriptor-bound at ~40ns/descriptor, so line length rules):
  - samples ship solo (s0, s1) or column-paired (4608B -> 9216B lines),
  - conv weights ship as two quad-t blocks (9216B lines),
  - every DRAM region touched by one dma_start is fully contiguous,
  - all small constants ride in ONE 128x680 bf16 blob (f32 views via
    bitcast), since each tiny transfer costs 128 descriptors (~2-3us of
    queue time) — two of them in front of x_0 cost 6us of startup,
  - the output is group-major [g, j, c, hw] so each blend group's slab
    store is one contiguous 1.18MB write (host transposes at gather);
    slab stores rotate across the sync/scalar HWDGE + gpsimd SWDGE rings.

PE order is hand-arranged (c0, c1, MLP+softmax, c2, BD, b0, c3, b1, b2,
c4, b3, c5, b4, c6, b5, b6, c7, b7) so the PE never waits on the
attention pipeline and only one blend remains after the last conv.
"""

import sys

import numpy as np

if "/opt/trn_rl_repo" not in sys.path:
    sys.path.insert(0, "/opt/trn_rl_repo")

import concourse.bacc as bacc
import concourse.bass as bass
import concourse.mybir as mybir
import concourse.tile as tile

F32 = mybir.dt.float32
BF16 = mybir.dt.bfloat16
AF = mybir.ActivationFunctionType
AX = mybir.AxisListType
ALU = mybir.AluOpType

B = 8
CIN = 128
COUT = 256
K = 4
KS = 3
HW = 48
HW2 = HW * HW          # 2304
WP = HW + 2            # 50 (padded)
HID = 256
TAU = 30.0
NCORES = 8

ROW_GROUPS = [(0, 10), (10, 10), (20, 10), (30, 10), (40, 8)]
CHUNKS = [(0, 480), (480, 480), (960, 480), (1440, 480), (1920, 384)]

# const blob column map (bf16 units)
CB_W1 = 0          # [128, 256] bf16
CB_CM = 256        # [128, 256] bf16 cmask
CB_BCT = 512       # [128, 16] bf16 view = [128, 8] f32 conv bias
CB_B1 = 528        # [128, 4] bf16 view = [128, 2] f32 mlp bias1
CB_K1 = 532        # [4, 128] bf16 one-hot replicate
CB_ID8 = 660       # [8, 8] bf16 identity
CB_B2 = 668        # [1, 4] bf16 mlp bias2
CB_ONE = 672       # [1, 8] bf16 ones
CB_W2 = 680        # [128, 8] bf16 w2 (two 128-col halves stacked)
CB_COLS = 688


def build_nc():
    nc = bacc.Bacc("TRN2", debug=False, num_devices=NCORES)

    # samples 0,1 solo (rows (s, ci)); samples 2..7 column-paired
    xsolo = nc.dram_tensor("xsolo", [2 * CIN, HW2], BF16, kind="ExternalInput").ap()
    xpack = nc.dram_tensor(
        "xpack", [3 * CIN, 2 * HW2], BF16, kind="ExternalInput"
    ).ap()
    # conv weights as four pair-t blocks: rows (q, ci), line (tq, tap, p)
    wconv2 = nc.dram_tensor(
        "wconv2", [4 * CIN, 2 * 9 * 128], BF16, kind="ExternalInput"
    ).ap()
    cblob = nc.dram_tensor("cblob", [128, CB_COLS], BF16, kind="ExternalInput").ap()
    # group-major output: [g, j, c, hw] — each [g] slab is contiguous
    out = nc.dram_tensor("out", [16, B, 16, HW2], F32, kind="ExternalOutput").ap()

    with tile.TileContext(nc, num_cores=NCORES) as tc:
        with (
            tc.tile_pool(name="const", bufs=1) as const,
            tc.tile_pool(name="csb", bufs=4) as csb_pool,
            tc.tile_pool(name="osb", bufs=5) as osb_pool,
            tc.tile_pool(name="psA", bufs=3, space="PSUM") as psA,
            tc.tile_pool(name="psB", bufs=4, space="PSUM") as psB,
            tc.tile_pool(name="psM", bufs=1, space="PSUM") as psM,
        ):
            # ---- DMA issue (sync: own sample + first weights; scalar:
            # consts + pairs; gpsimd: pair + second weights) ----
            wtq = []
            for q in range(4):
                w = const.tile([128, 2 * 9 * 128], BF16, tag=f"wtq{q}")
                wtq.append(w)
            xs0 = const.tile([128, HW2], BF16)
            nc.sync.dma_start(xs0[:], xsolo[0:128, :])
            xs1 = const.tile([128, HW2], BF16)
            nc.sync.dma_start(xs1[:], xsolo[128:256, :])
            nc.sync.dma_start(wtq[1][:], wconv2[128:256, :])
            nc.sync.dma_start(wtq[3][:], wconv2[384:512, :])

            nc.scalar.dma_start(wtq[0][:], wconv2[0:128, :])
            blob = const.tile([128, CB_COLS], BF16)
            nc.scalar.dma_start(blob[:], cblob[:, :])
            xp23 = const.tile([128, 2 * HW2], BF16)
            nc.scalar.dma_start(xp23[:], xpack[0:128, :])

            xp45 = const.tile([128, 2 * HW2], BF16)
            nc.gpsimd.dma_start(xp45[:], xpack[128:256, :])
            xp67 = const.tile([128, 2 * HW2], BF16)
            nc.gpsimd.dma_start(xp67[:], xpack[256:384, :])
            nc.gpsimd.dma_start(wtq[2][:], wconv2[256:384, :])

            # const views out of the blob
            w1s = blob[:, CB_W1 : CB_W1 + 256]
            cms = blob[:, CB_CM : CB_CM + 256]
            bct = blob[:, CB_BCT : CB_BCT + 16].bitcast(F32)
            b1s = blob[:, CB_B1 : CB_B1 + 4].bitcast(F32)
            k1s = blob[0:K, CB_K1 : CB_K1 + 128]
            id8 = blob[0:B, CB_ID8 : CB_ID8 + 8]
            b2s = blob[0:1, CB_B2 : CB_B2 + K]
            ones = blob[0:1, CB_ONE : CB_ONE + B]
            w2s = blob[:, CB_W2 : CB_W2 + 2 * K]

            # ACT table pre-warm off a memset zero column
            zf = const.tile([128, 1], F32)
            nc.gpsimd.memset(zf[:], 0.0)
            actw = const.tile([128, 1], F32)
            zcol = zf[:, 0:1]
            nc.scalar.activation(actw[:], zcol, AF.Identity, bias=zcol)
            nc.scalar.activation(actw[:], zcol, AF.Relu, bias=zcol)
            nc.scalar.activation(actw[:], zcol, AF.Exp, bias=zcol)
            nc.scalar.copy(actw[:], zcol)

            # padded image: borders memset (gpsimd), interior copied (DVE)
            xp = const.tile([128, WP * WP], BF16)
            xp3 = xp[:].rearrange("p (h w) -> p h w", w=WP)
            nc.gpsimd.memset(xp3[:, 0, 0:WP], 0.0)
            nc.gpsimd.memset(xp3[:, WP - 1, 0:WP], 0.0)
            nc.gpsimd.memset(xp3[:, 1 : 1 + HW, 0], 0.0)
            nc.gpsimd.memset(xp3[:, 1 : 1 + HW, WP - 1], 0.0)
            x03 = xs0[:].rearrange("p (h w) -> p h w", w=HW)
            nc.vector.tensor_copy(xp3[:, 1 : 1 + HW, 1 : 1 + HW], x03[:, :, :])

            # global-average pools, pipelined with arrival; samples 0-5
            # reduce on DVE, 6-7 on ACT (activation accum_out) so the two
            # chains overlap; 1/HW2 is folded into w1 on the host
            pooled = const.tile([128, B], F32)
            nc.gpsimd.memset(pooled[:], 0.0)
            nc.vector.tensor_reduce(pooled[:, 0:1], xs0[:], axis=AX.X, op=ALU.add)
            nc.vector.tensor_reduce(pooled[:, 1:2], xs1[:], axis=AX.X, op=ALU.add)
            for s0_, tl in ((2, xp23), (4, xp45)):
                v = tl[:].rearrange("p (s f) -> p s f", s=2)
                nc.vector.tensor_reduce(
                    pooled[:, s0_ : s0_ + 2], v, axis=AX.X, op=ALU.add
                )
            cs_tiles = [None] * 8

            def emit_conv(t):
                cs = csb_pool.tile([128, HW2], BF16, tag="csb")
                cs_tiles[t] = cs
                wq = wtq[t // 2]
                base = (t % 2) * 9 * 128
                for (r0, R) in ROW_GROUPS:
                    pt = psA.tile([128, R * HW], F32, tag="cps")
                    for tap in range(9):
                        dh, dw = divmod(tap, 3)
                        rhs = xp3[:, r0 + dh : r0 + dh + R, dw : dw + HW]
                        nc.tensor.matmul(
                            pt[:],
                            lhsT=wq[:, base + tap * 128 : base + (tap + 1) * 128],
                            rhs=rhs,
                            start=(tap == 0),
                            stop=(tap == 8),
                        )
                    # PSUM -> SBUF eviction, fused with the conv bias add
                    nc.scalar.activation(
                        cs[:, r0 * HW : (r0 + R) * HW],
                        pt[:],
                        AF.Identity,
                        bias=bct[:, t : t + 1],
                    )

            dma_rr = [0]

            def emit_blend(t):
                cs = cs_tiles[t]
                for u in range(2):
                    g = 2 * t + u
                    ob = osb_pool.tile([128, HW2], F32, tag="osb")
                    for ci_, (c0, C) in enumerate(CHUNKS):
                        bp = psB.tile([128, C], F32, tag="bps")
                        nc.tensor.matmul(
                            bp[:],
                            lhsT=bd[:, 128 * u : 128 * u + 128],
                            rhs=cs[:, c0 : c0 + C],
                            start=True,
                            stop=True,
                        )
                        # PSUM drain balanced across DVE and ACT so psB bank
                        # recycling (not one engine) sets the blend rate
                        if ci_ in (1, 4):
                            nc.scalar.copy(ob[:, c0 : c0 + C], bp[:])
                        else:
                            nc.vector.tensor_copy(ob[:, c0 : c0 + C], bp[:])
                    # column-half stores on two rotating rings (per-ring
                    # throughput caps at ~118GB/s; halves recycle ob faster)
                    ov = out[g, :, :, :].rearrange("j c (h f) -> j c h f", h=2)
                    obv = ob[:].rearrange("p (h f) -> p h f", h=2)
                    for h in range(2):
                        eng = (nc.sync, nc.scalar, nc.gpsimd)[dma_rr[0] % 3]
                        dma_rr[0] += 1
                        eng.dma_start(ov[:, :, h], obv[:, h])

            emit_conv(0)

            # ACT-side pools for samples 6,7 (junk main output; the
            # per-partition accumulator is the value we want)
            junk = const.tile([128, HW2], BF16)
            xp67v = xp67[:].rearrange("p (s f) -> p s f", s=2)
            nc.scalar.activation(
                junk[:], xp67v[:, 0, :], AF.Identity, accum_out=pooled[:, 6:7]
            )
            nc.scalar.activation(
                junk[:], xp67v[:, 1, :], AF.Identity, accum_out=pooled[:, 7:8]
            )
            pooled8 = const.tile([128, B], BF16)
            nc.vector.tensor_copy(pooled8[:], pooled[:])

            emit_conv(1)

            # ---- attention MLP over all 8 pooled rows ----
            hd = []
            for h in range(2):
                hps = psM.tile([128, B], F32, tag="mlp")
                nc.tensor.matmul(
                    hps[:],
                    lhsT=w1s[:, h * 128 : (h + 1) * 128],
                    rhs=pooled8[:],
                    start=True,
                    stop=True,
                )
                hsb = const.tile([128, B], BF16, tag=f"hd{h}")
                nc.scalar.activation(hsb[:], hps[:], AF.Relu, bias=b1s[:, h : h + 1])
                hd.append(hsb)

            lps = psM.tile([B, K], F32, tag="mlp")
            nc.tensor.matmul(
                lps[:], lhsT=hd[0][:], rhs=w2s[:, 0:K], start=True, stop=False
            )
            nc.tensor.matmul(
                lps[:], lhsT=hd[1][:], rhs=w2s[:, K : 2 * K], start=False, stop=False
            )
            nc.tensor.matmul(
                lps[:], lhsT=ones, rhs=b2s, start=False, stop=True
            )

            # double softmax over k (shift-invariant: max-subtraction dropped)
            e1 = const.tile([B, K], F32)
            nc.scalar.activation(e1[:], lps[:], AF.Exp, bias=0.0, scale=1.0)
            s1 = const.tile([B, 1], F32)
            nc.vector.tensor_reduce(s1[:], e1[:], axis=AX.X, op=ALU.add)
            r1 = const.tile([B, 1], F32)
            nc.vector.reciprocal(r1[:], s1[:])
            a1 = const.tile([B, K], F32)
            nc.vector.tensor_scalar_mul(a1[:], e1[:], r1[:, 0:1])

            e2 = const.tile([B, K], F32)
            nc.scalar.activation(e2[:], a1[:], AF.Exp, bias=0.0, scale=1.0 / TAU)
            s2 = const.tile([B, 1], F32)
            nc.vector.tensor_reduce(s2[:], e2[:], axis=AX.X, op=ALU.add)
            r2 = const.tile([B, 1], F32)
            nc.vector.reciprocal(r2[:], s2[:])
            attn = const.tile([B, K], BF16)
            nc.vector.tensor_scalar_mul(attn[:], e2[:], r2[:, 0:1])

            emit_conv(2)

            # ---- blend lhsT: BD[p, u*128 + j*16 + c] = attn[j, k] at
            # partition p = 64u + 4c + k, zero elsewhere ----
            tps = psM.tile([K, B], BF16, tag="mlp")
            nc.tensor.transpose(tps[:], attn[:], id8)
            atT = const.tile([K, B], BF16)
            nc.scalar.copy(atT[:], tps[:])
            # T1[p, j] = atT[p % 4, j]  (one-hot replicate matmul)
            t1p = psM.tile([128, B], F32, tag="mlp")
            nc.tensor.matmul(t1p[:], lhsT=k1s, rhs=atT[:], start=True, stop=True)
            t1 = const.tile([128, B], BF16)
            nc.scalar.copy(t1[:], t1p[:])
            # BD = T1 (broadcast over (u, c)) * CMASK
            bd = const.tile([128, 256], BF16)
            bdv = bd[:].rearrange("p (u j c) -> p u j c", u=2, c=16)
            cmv = cms.rearrange("p (u j c) -> p u j c", u=2, c=16)
            t1b = t1[:].rearrange("p (j c) -> p j c", c=1).broadcast_to([128, B, 16])
            for u in range(2):
                nc.vector.tensor_mul(bdv[:, u], cmv[:, u], t1b)

            # steady state; catch blends up so only b7 trails conv7
            emit_blend(0)
            emit_conv(3)
            emit_blend(1)
            emit_blend(2)
            emit_conv(4)
            emit_blend(3)
            emit_conv(5)
            emit_blend(4)
            emit_conv(6)
            emit_blend(5)
            emit_blend(6)
            emit_conv(7)
            emit_blend(7)

    nc.compile()
    return nc


def pack_inputs(x, conv_w, conv_b, w1, b1, w2, b2):
    """Host-side layout/dtype packing (no input-dependent arithmetic beyond
    constant folding of the mean-pool scale into w1)."""
    import ml_dtypes

    bf16 = ml_dtypes.bfloat16
    x = np.ascontiguousarray(x, dtype=np.float32)
    x_bf = x.reshape(B, CIN, HW2).astype(bf16)

    # conv_w [K, COUT, CIN, 3, 3] -> [ci, t, tap, p] with p = c*4 + k,
    # co = 32 t + c; then regrouped into two quad-t blocks (q, ci, tq, tap, p)
    w = np.asarray(conv_w, dtype=np.float32).transpose(2, 3, 4, 0, 1)  # ci kh kw k co
    w = w.reshape(CIN, KS, KS, K, 8, 32)  # ci kh kw k t c
    w = w.transpose(0, 4, 1, 2, 5, 3)  # ci t kh kw c k
    wfull = w.reshape(CIN, 8, 9 * 128)  # ci t (tap p)
    wconv2 = np.ascontiguousarray(
        wfull.reshape(CIN, 4, 2, 9 * 128).transpose(1, 0, 2, 3).reshape(
            4 * CIN, 2 * 9 * 128
        )
    ).astype(bf16)

    bc = np.asarray(conv_b, dtype=np.float32).reshape(K, 8, 32)  # k t c
    bconv = np.ascontiguousarray(bc.transpose(1, 2, 0).reshape(8, 128).T)  # [p, t]

    w1t = (np.ascontiguousarray(np.asarray(w1, dtype=np.float32).T) / float(HW2)).astype(bf16)
    b1c = np.ascontiguousarray(np.asarray(b1, dtype=np.float32).reshape(2, 128).T)
    w2T = np.asarray(w2, dtype=np.float32).T  # [256, 4]
    w2t = np.ascontiguousarray(np.concatenate([w2T[:128], w2T[128:]], axis=1)).astype(bf16)

    p = np.arange(128)
    col = np.arange(256)
    cmask = (
        ((col[None, :] // 128) == (p[:, None] // 64))
        & ((col[None, :] % 16) == ((p[:, None] % 64) // 4))
    ).astype(bf16)
    k1m = (np.arange(K)[:, None] == (p[None, :] % 4)).astype(bf16)

    # single constant blob [128, CB_COLS] bf16 (f32 values bitcast into pairs)
    cb = np.zeros((128, CB_COLS), dtype=bf16)
    cb[:, CB_W1 : CB_W1 + 256] = w1t
    cb[:, CB_CM : CB_CM + 256] = cmask
    cb[:, CB_BCT : CB_BCT + 16] = bconv.astype(np.float32).view(bf16)[:, : 16]
    cb[:, CB_B1 : CB_B1 + 4] = b1c.astype(np.float32).view(bf16)[:, : 4]
    cb[0:K, CB_K1 : CB_K1 + 128] = k1m
    cb[0:B, CB_ID8 : CB_ID8 + 8] = np.eye(B, dtype=bf16)
    cb[0:1, CB_B2 : CB_B2 + K] = np.asarray(b2, dtype=np.float32).reshape(1, K).astype(bf16)
    cb[0:1, CB_ONE : CB_ONE + B] = np.ones((1, B), dtype=bf16)
    cb[:, CB_W2 : CB_W2 + 2 * K] = w2t

    common = dict(wconv2=wconv2, cblob=cb)
    in_maps = []
    for i in range(NCORES):
        # rotate so core i's own sample is slice 0
        xr = [x_bf[(i + r) % B] for r in range(B)]
        xsolo = np.ascontiguousarray(np.concatenate([xr[0], xr[1]], axis=0))
        xpack = np.ascontiguousarray(
            np.concatenate(
                [np.concatenate([xr[2 * q], xr[2 * q + 1]], axis=1) for q in (1, 2, 3)],
                axis=0,
            )
        )
        in_maps.append(dict(common, xsolo=xsolo, xpack=xpack))
    return in_maps


def run(inputs, trace=False):
    from concourse.bass_utils import run_bass_kernel_spmd

    nc = build_nc()
    in_maps = pack_inputs(**inputs)
    res = run_bass_kernel_spmd(
        nc, in_maps, core_ids=list(range(NCORES)), trace=trace
    )
    out = np.empty((B, B, COUT, HW2), dtype=np.float32)
    for i in range(NCORES):
        slab = res.results[i]["out"]  # [g, j(rot), c, hw]
        slab = slab.transpose(1, 0, 2, 3).reshape(B, COUT, HW2)
        out[i] = slab[(np.arange(B) - i) % B]
    return out.reshape(B, B, COUT, HW, HW), res


def kernel(**inputs) -> np.ndarray:
    out, _ = run(inputs, trace=False)
    return out


# revision 11
# speedup vs baseline: 1.4829x; 1.1020x over previous
"""Trainium2 Bass kernel for nn_DynamicConv (dense_cnn).

out[i, j, co, h, w] = sum_k (conv_k(x_i)[co, h, w] + b_k[co]) * attn[j, k]
attn = softmax(softmax(MLP(meanpool(x)), k) / TAU, k)

Sharding: data-parallel over batch i across 8 cores.  All matmuls run in
bf16 (fp32r lowers to fp32_mode=HIGH at 2 cycles/column — half PE rate;
bf16 streams 1 col/cycle, ~203ns per N=480 matmul with per-MM LDWEIGHTS
fully hidden).

No collective: every core receives ALL of x in bf16, rotated so its own
sample is sample 0, and computes the full [B, K] attention matrix
locally; the host un-rotates the j rows of each core's output slab at
gather time.  (The AllGather path costs ~50us wall for 16 bytes.)

DMA layout rules learned from traces (per-queue throughput is
descriptor-bound at ~40ns/descriptor, so line length rules):
  - samples ship solo (s0, s1) or column-paired (4608B -> 9216B lines),
  - conv weights ship as two quad-t blocks (9216B lines),
  - every DRAM region touched by one dma_start is fully contiguous,
  - all small constants ride in ONE 128x680 bf16 blob (f32 views via
    bitcast), since each tiny transfer costs 128 descriptors (~2-3us of
    queue time) — two of them in front of x_0 cost 6us of startup,
  - the output is group-major [g, j, c, hw] so each blend group's slab
    store is one contiguous 1.18MB write (host transposes at gather);
    slab stores rotate across the sync/scalar HWDGE + gpsimd SWDGE rings.

PE order is hand-arranged (c0, c1, MLP+softmax, c2, BD, b0, c3, b1, b2,
c4, b3, c5, b4, c6, b5, b6, c7, b7) so the PE never waits on the
attention pipeline and only one blend remains after the last conv.
"""

import sys

import numpy as np

if "/opt/trn_rl_repo" not in sys.path:
    sys.path.insert(0, "/opt/trn_rl_repo")

import concourse.bacc as bacc
import concourse.bass as bass
import concourse.mybir as mybir
import concourse.tile as tile

F32 = mybir.dt.float32
BF16 = mybir.dt.bfloat16
F8 = mybir.dt.float8e4
AF = mybir.ActivationFunctionType
AX = mybir.AxisListType
ALU = mybir.AluOpType

B = 8
CIN = 128
COUT = 256
K = 4
KS = 3
HW = 48
HW2 = HW * HW          # 2304
WP = HW + 2            # 50 (padded)
HID = 256
TAU = 30.0
NCORES = 8

ROW_GROUPS = [(0, 10), (10, 10), (20, 10), (30, 10), (40, 8)]
CHUNKS = [(0, 480), (480, 480), (960, 480), (1440, 480), (1920, 384)]

# const blob column map (bf16 units)
CB_W1 = 0          # [128, 256] bf16
CB_CM = 256        # [128, 256] bf16 cmask
CB_BCT = 512       # [128, 16] bf16 view = [128, 8] f32 conv bias
CB_B1 = 528        # [128, 4] bf16 view = [128, 2] f32 mlp bias1
CB_K1 = 532        # [4, 128] bf16 one-hot replicate
CB_ID8 = 660       # [8, 8] bf16 identity
CB_B2 = 668        # [1, 4] bf16 mlp bias2
CB_ONE = 672       # [1, 8] bf16 ones
CB_W2 = 680        # [128, 8] bf16 w2 (two 128-col halves stacked)
CB_COLS = 688


def build_nc():
    nc = bacc.Bacc("TRN2", debug=False, num_devices=NCORES)

    # own sample bf16 (convolved); samples 1-7 fp8, pool-only,
    # column-packed so DMA lines stay long and contiguous
    xown = nc.dram_tensor("xown", [CIN, HW2], BF16, kind="ExternalInput").ap()
    x123 = nc.dram_tensor("x123", [CIN, 3 * HW2], F8, kind="ExternalInput").ap()
    x4567 = nc.dram_tensor("x4567", [CIN, 4 * HW2], F8, kind="ExternalInput").ap()
    # conv weights as four pair-t blocks: rows (q, ci), line (tq, tap, p)
    wconv2 = nc.dram_tensor(
        "wconv2", [4 * CIN, 2 * 9 * 128], BF16, kind="ExternalInput"
    ).ap()
    cblob = nc.dram_tensor("cblob", [128, CB_COLS], BF16, kind="ExternalInput").ap()
    # group-major output: [g, j, c, hw] — each [g] slab is contiguous
    out = nc.dram_tensor("out", [16, B, 16, HW2], F32, kind="ExternalOutput").ap()

    with tile.TileContext(nc, num_cores=NCORES) as tc:
        with (
            tc.tile_pool(name="const", bufs=1) as const,
            tc.tile_pool(name="csb", bufs=4) as csb_pool,
            tc.tile_pool(name="osb", bufs=5) as osb_pool,
            tc.tile_pool(name="psA", bufs=3, space="PSUM") as psA,
            tc.tile_pool(name="psB", bufs=4, space="PSUM") as psB,
            tc.tile_pool(name="psM", bufs=1, space="PSUM") as psM,
        ):
            # ---- DMA issue (sync: own sample + first weights; scalar:
            # consts + pairs; gpsimd: pair + second weights) ----
            wtq = []
            for q in range(4):
                w = const.tile([128, 2 * 9 * 128], BF16, tag=f"wtq{q}")
                wtq.append(w)
            xs0 = const.tile([128, HW2], BF16)
            nc.sync.dma_start(xs0[:], xown[:, :])
            xo123 = const.tile([128, 3 * HW2], F8)
            nc.sync.dma_start(xo123[:], x123[:, :])
            nc.sync.dma_start(wtq[1][:], wconv2[128:256, :])

            nc.scalar.dma_start(wtq[0][:], wconv2[0:128, :])
            blob = const.tile([128, CB_COLS], BF16)
            nc.scalar.dma_start(blob[:], cblob[:, :])
            nc.scalar.dma_start(wtq[3][:], wconv2[384:512, :])

            xo4567 = const.tile([128, 4 * HW2], F8)
            nc.gpsimd.dma_start(xo4567[:], x4567[:, :])
            nc.gpsimd.dma_start(wtq[2][:], wconv2[256:384, :])

            # const views out of the blob
            w1s = blob[:, CB_W1 : CB_W1 + 256]
            cms = blob[:, CB_CM : CB_CM + 256]
            bct = blob[:, CB_BCT : CB_BCT + 16].bitcast(F32)
            b1s = blob[:, CB_B1 : CB_B1 + 4].bitcast(F32)
            k1s = blob[0:K, CB_K1 : CB_K1 + 128]
            id8 = blob[0:B, CB_ID8 : CB_ID8 + 8]
            b2s = blob[0:1, CB_B2 : CB_B2 + K]
            ones = blob[0:1, CB_ONE : CB_ONE + B]
            w2s = blob[:, CB_W2 : CB_W2 + 2 * K]

            # ACT table pre-warm off a memset zero column
            zf = const.tile([128, 1], F32)
            nc.gpsimd.memset(zf[:], 0.0)
            actw = const.tile([128, 1], F32)
            zcol = zf[:, 0:1]
            nc.scalar.activation(actw[:], zcol, AF.Identity, bias=zcol)
            nc.scalar.activation(actw[:], zcol, AF.Relu, bias=zcol)
            nc.scalar.activation(actw[:], zcol, AF.Exp, bias=zcol)
            nc.scalar.copy(actw[:], zcol)

            # padded image: borders memset (gpsimd), interior copied (DVE)
            xp = const.tile([128, WP * WP], BF16)
            xp3 = xp[:].rearrange("p (h w) -> p h w", w=WP)
            nc.gpsimd.memset(xp3[:, 0, 0:WP], 0.0)
            nc.gpsimd.memset(xp3[:, WP - 1, 0:WP], 0.0)
            nc.gpsimd.memset(xp3[:, 1 : 1 + HW, 0], 0.0)
            nc.gpsimd.memset(xp3[:, 1 : 1 + HW, WP - 1], 0.0)
            x03 = xs0[:].rearrange("p (h w) -> p h w", w=HW)
            nc.vector.tensor_copy(xp3[:, 1 : 1 + HW, 1 : 1 + HW], x03[:, :, :])

            # global-average pools (all DVE, pipelined with arrival);
            # samples 1-7 are fp8 — the double-softmax(/TAU) attenuates
            # pooled error by ~1000x so fp8 pooling is harmless;
            # 1/HW2 is folded into w1 on the host
            pooled = const.tile([128, B], F32)
            nc.vector.tensor_reduce(pooled[:, 0:1], xs0[:], axis=AX.X, op=ALU.add)
            v123 = xo123[:].rearrange("p (s f) -> p s f", s=3)
            nc.vector.tensor_reduce(pooled[:, 1:4], v123, axis=AX.X, op=ALU.add)
            v4567 = xo4567[:].rearrange("p (s f) -> p s f", s=4)
            nc.vector.tensor_reduce(pooled[:, 4:8], v4567, axis=AX.X, op=ALU.add)
            cs_tiles = [None] * 8

            def emit_conv(t):
                cs = csb_pool.tile([128, HW2], BF16, tag="csb")
                cs_tiles[t] = cs
                wq = wtq[t // 2]
                base = (t % 2) * 9 * 128
                for (r0, R) in ROW_GROUPS:
                    pt = psA.tile([128, R * HW], F32, tag="cps")
                    for tap in range(9):
                        dh, dw = divmod(tap, 3)
                        rhs = xp3[:, r0 + dh : r0 + dh + R, dw : dw + HW]
                        nc.tensor.matmul(
                            pt[:],
                            lhsT=wq[:, base + tap * 128 : base + (tap + 1) * 128],
                            rhs=rhs,
                            start=(tap == 0),
                            stop=(tap == 8),
                        )
                    # PSUM -> SBUF eviction, fused with the conv bias add
                    nc.scalar.activation(
                        cs[:, r0 * HW : (r0 + R) * HW],
                        pt[:],
                        AF.Identity,
                        bias=bct[:, t : t + 1],
                    )

            dma_rr = [0]

            def emit_blend(t):
                cs = cs_tiles[t]
                for u in range(2):
                    g = 2 * t + u
                    ob = osb_pool.tile([128, HW2], F32, tag="osb")
                    for ci_, (c0, C) in enumerate(CHUNKS):
                        bp = psB.tile([128, C], F32, tag="bps")
                        nc.tensor.matmul(
                            bp[:],
                            lhsT=bd[:, 128 * u : 128 * u + 128],
                            rhs=cs[:, c0 : c0 + C],
                            start=True,
                            stop=True,
                        )
                        # PSUM drain balanced across DVE and ACT so psB bank
                        # recycling (not one engine) sets the blend rate
                        if ci_ in (1, 4):
                            nc.scalar.copy(ob[:, c0 : c0 + C], bp[:])
                        else:
                            nc.vector.tensor_copy(ob[:, c0 : c0 + C], bp[:])
                    # column-half stores on two rotating rings (per-ring
                    # throughput caps at ~118GB/s; halves recycle ob faster)
                    ov = out[g, :, :, :].rearrange("j c (h f) -> j c h f", h=2)
                    obv = ob[:].rearrange("p (h f) -> p h f", h=2)
                    for h in range(2):
                        eng = (nc.sync, nc.scalar, nc.gpsimd)[dma_rr[0] % 3]
                        dma_rr[0] += 1
                        eng.dma_start(ov[:, :, h], obv[:, h])

            pooled8 = const.tile([128, B], BF16)
            nc.vector.tensor_copy(pooled8[:], pooled[:])

            emit_conv(0)
            emit_conv(1)
            emit_conv(2)

            # ---- attention MLP over all 8 pooled rows ----
            hd = []
            for h in range(2):
                hps = psM.tile([128, B], F32, tag="mlp")
                nc.tensor.matmul(
                    hps[:],
                    lhsT=w1s[:, h * 128 : (h + 1) * 128],
                    rhs=pooled8[:],
                    start=True,
                    stop=True,
                )
                hsb = const.tile([128, B], BF16, tag=f"hd{h}")
                nc.scalar.activation(hsb[:], hps[:], AF.Relu, bias=b1s[:, h : h + 1])
                hd.append(hsb)

            lps = psM.tile([B, K], F32, tag="mlp")
            nc.tensor.matmul(
                lps[:], lhsT=hd[0][:], rhs=w2s[:, 0:K], start=True, stop=False
            )
            nc.tensor.matmul(
                lps[:], lhsT=hd[1][:], rhs=w2s[:, K : 2 * K], start=False, stop=False
            )
            nc.tensor.matmul(
                lps[:], lhsT=ones, rhs=b2s, start=False, stop=True
            )

            # double softmax over k (shift-invariant: max-subtraction dropped)
            e1 = const.tile([B, K], F32)
            nc.scalar.activation(e1[:], lps[:], AF.Exp, bias=0.0, scale=1.0)
            s1 = const.tile([B, 1], F32)
            nc.vector.tensor_reduce(s1[:], e1[:], axis=AX.X, op=ALU.add)
            r1 = const.tile([B, 1], F32)
            nc.vector.reciprocal(r1[:], s1[:])
            a1 = const.tile([B, K], F32)
            nc.vector.tensor_scalar_mul(a1[:], e1[:], r1[:, 0:1])

            e2 = const.tile([B, K], F32)
            nc.scalar.activation(e2[:], a1[:], AF.Exp, bias=0.0, scale=1.0 / TAU)
            s2 = const.tile([B, 1], F32)
            nc.vector.tensor_reduce(s2[:], e2[:], axis=AX.X, op=ALU.add)
            r2 = const.tile([B, 1], F32)
            nc.vector.reciprocal(r2[:], s2[:])
            attn = const.tile([B, K], BF16)
            nc.vector.tensor_scalar_mul(attn[:], e2[:], r2[:, 0:1])

            # ---- blend lhsT: BD[p, u*128 + j*16 + c] = attn[j, k] at
            # partition p = 64u + 4c + k, zero elsewhere ----
            tps = psM.tile([K, B], BF16, tag="mlp")
            nc.tensor.transpose(tps[:], attn[:], id8)
            atT = const.tile([K, B], BF16)
            nc.scalar.copy(atT[:], tps[:])
            # T1[p, j] = atT[p % 4, j]  (one-hot replicate matmul)
            t1p = psM.tile([128, B], F32, tag="mlp")
            nc.tensor.matmul(t1p[:], lhsT=k1s, rhs=atT[:], start=True, stop=True)
            t1 = const.tile([128, B], BF16)
            nc.scalar.copy(t1[:], t1p[:])
            # BD = T1 (broadcast over (u, c)) * CMASK
            bd = const.tile([128, 256], BF16)
            bdv = bd[:].rearrange("p (u j c) -> p u j c", u=2, c=16)
            cmv = cms.rearrange("p (u j c) -> p u j c", u=2, c=16)
            t1b = t1[:].rearrange("p (j c) -> p j c", c=1).broadcast_to([128, B, 16])
            for u in range(2):
                nc.vector.tensor_mul(bdv[:, u], cmv[:, u], t1b)

            # steady state; catch blends up so only b7 trails conv7
            emit_blend(0)
            emit_conv(3)
            emit_blend(1)
            emit_blend(2)
            emit_conv(4)
            emit_blend(3)
            emit_conv(5)
            emit_blend(4)
            emit_conv(6)
            emit_blend(5)
            emit_blend(6)
            emit_conv(7)
            emit_blend(7)

    nc.compile()
    return nc


def pack_inputs(x, conv_w, conv_b, w1, b1, w2, b2):
    """Host-side layout/dtype packing (no input-dependent arithmetic beyond
    constant folding of the mean-pool scale into w1)."""
    import ml_dtypes

    bf16 = ml_dtypes.bfloat16
    x = np.ascontiguousarray(x, dtype=np.float32)
    x_bf = x.reshape(B, CIN, HW2).astype(bf16)

    # conv_w [K, COUT, CIN, 3, 3] -> [ci, t, tap, p] with p = c*4 + k,
    # co = 32 t + c; then regrouped into two quad-t blocks (q, ci, tq, tap, p)
    w = np.asarray(conv_w, dtype=np.float32).transpose(2, 3, 4, 0, 1)  # ci kh kw k co
    w = w.reshape(CIN, KS, KS, K, 8, 32)  # ci kh kw k t c
    w = w.transpose(0, 4, 1, 2, 5, 3)  # ci t kh kw c k
    wfull = w.reshape(CIN, 8, 9 * 128)  # ci t (tap p)
    wconv2 = np.ascontiguousarray(
        wfull.reshape(CIN, 4, 2, 9 * 128).transpose(1, 0, 2, 3).reshape(
            4 * CIN, 2 * 9 * 128
        )
    ).astype(bf16)

    bc = np.asarray(conv_b, dtype=np.float32).reshape(K, 8, 32)  # k t c
    bconv = np.ascontiguousarray(bc.transpose(1, 2, 0).reshape(8, 128).T)  # [p, t]

    w1t = (np.ascontiguousarray(np.asarray(w1, dtype=np.float32).T) / float(HW2)).astype(bf16)
    b1c = np.ascontiguousarray(np.asarray(b1, dtype=np.float32).reshape(2, 128).T)
    w2T = np.asarray(w2, dtype=np.float32).T  # [256, 4]
    w2t = np.ascontiguousarray(np.concatenate([w2T[:128], w2T[128:]], axis=1)).astype(bf16)

    p = np.arange(128)
    col = np.arange(256)
    cmask = (
        ((col[None, :] // 128) == (p[:, None] // 64))
        & ((col[None, :] % 16) == ((p[:, None] % 64) // 4))
    ).astype(bf16)
    k1m = (np.arange(K)[:, None] == (p[None, :] % 4)).astype(bf16)

    # single constant blob [128, CB_COLS] bf16 (f32 values bitcast into pairs)
    cb = np.zeros((128, CB_COLS), dtype=bf16)
    cb[:, CB_W1 : CB_W1 + 256] = w1t
    cb[:, CB_CM : CB_CM + 256] = cmask
    cb[:, CB_BCT : CB_BCT + 16] = bconv.astype(np.float32).view(bf16)[:, : 16]
    cb[:, CB_B1 : CB_B1 + 4] = b1c.astype(np.float32).view(bf16)[:, : 4]
    cb[0:K, CB_K1 : CB_K1 + 128] = k1m
    cb[0:B, CB_ID8 : CB_ID8 + 8] = np.eye(B, dtype=bf16)
    cb[0:1, CB_B2 : CB_B2 + K] = np.asarray(b2, dtype=np.float32).reshape(1, K).astype(bf16)
    cb[0:1, CB_ONE : CB_ONE + B] = np.ones((1, B), dtype=bf16)
    cb[:, CB_W2 : CB_W2 + 2 * K] = w2t

    f8 = mybir.dt.np(F8)
    x_f8 = x.reshape(B, CIN, HW2).astype(f8)

    common = dict(wconv2=wconv2, cblob=cb)
    in_maps = []
    for i in range(NCORES):
        # rotate so core i's own sample is slice 0; samples 1-7 ship as
        # fp8 (pool-only), column-packed for long contiguous DMA lines
        ids = [(i + r) % B for r in range(B)]
        xown = np.ascontiguousarray(x_bf[ids[0]])
        x123 = np.ascontiguousarray(
            np.concatenate([x_f8[s] for s in ids[1:4]], axis=1)
        )
        x4567 = np.ascontiguousarray(
            np.concatenate([x_f8[s] for s in ids[4:8]], axis=1)
        )
        in_maps.append(dict(common, xown=xown, x123=x123, x4567=x4567))
    return in_maps


def run(inputs, trace=False):
    from concourse.bass_utils import run_bass_kernel_spmd

    nc = build_nc()
    in_maps = pack_inputs(**inputs)
    res = run_bass_kernel_spmd(
        nc, in_maps, core_ids=list(range(NCORES)), trace=trace
    )
    out = np.empty((B, B, COUT, HW2), dtype=np.float32)
    for i in range(NCORES):
        slab = res.results[i]["out"]  # [g, j(rot), c, hw]
        slab = slab.transpose(1, 0, 2, 3).reshape(B, COUT, HW2)
        out[i] = slab[(np.arange(B) - i) % B]
    return out.reshape(B, B, COUT, HW, HW), res


def kernel(**inputs) -> np.ndarray:
    out, _ = run(inputs, trace=False)
    return out
